# revision 19
# baseline (speedup 1.0000x reference)
"""Trainium2 Bass kernel for a 2-layer GCN (EnhancedGNN) with triple global
pooling and a final FC, run SPMD across 8 NeuronCores.

Strategy v3:
  - Nodes are re-ordered into graph-pure 128-row blocks, padded per graph,
    sharded contiguously across the 8 cores (dst / data parallel). Within
    each core, blocks are sorted by edge count so the max-over-cores tile
    padding shrinks (the SPMD program uses per-(block,half) maxima).
  - Layer 1 does NOT gather on device: since X is a replicated input and
    scatter/transform commute ((B^T Xg) @ W1 == B^T (Xg @ W1)), the host
    pre-lays-out dinv-scaled X rows in message-tile order and the device
    streams them with affine DMA, scatters them into per-dst-block PSUM via
    one-hot matmuls, then applies W1 (and W2 to build the layer-2 table).
  - Self-loop messages fold in as an identity matmul against the block's
    own rows (X rows for layer 1, own layer-2 table rows for layer 2).
  - The layer-2 table AllGather is chunked (NCHUNK pieces) so all but the
    last chunk overlap layer-1 compute; a tiny warmup AllReduce at program
    start absorbs the collectives' entry barrier.
  - Layer 2: per-(block,half) dma_gather calls with trailing -1 padding and
    a per-core runtime num_idxs_reg, so the GpSimd Q7 descriptor generation
    (the measured bottleneck) only pays for real edges on each core.
  - Pooling: per-graph sums via one-hot matmul; per-graph max accumulated
    incrementally per block (vector max) during layer 2; AllReduce(add/max);
    the tiny FC runs redundantly on every core.

The kernel program is identical on all 8 cores (SPMD); all per-core
differences live in the input data. Structure constants (tile counts etc.)
are maxima over cores so the program is uniform.
"""

import os
import numpy as np
import ml_dtypes

import concourse.bass as bass
import concourse.tile as tile
from concourse import bacc, mybir
from concourse.bass_utils import run_bass_kernel_spmd

P = 128
NCORES = 8
GROUP_NBLK = 2  # dst blocks per gather/stream group

BF16 = ml_dtypes.bfloat16


def _cdiv(a, b):
    return -(-a // b)


# --------------------------------------------------------------------------
# Host-side preprocessing: sharding, edge grouping, auxiliary tensors.
# --------------------------------------------------------------------------

def preprocess(x, edge_index, batch, n_graphs, W1, b1, W2, b2, Wfc, bfc,
               n_cores=NCORES):
    x = np.asarray(x, np.float32)
    ei = np.asarray(edge_index, np.int64)
    batch = np.asarray(batch, np.int64)
    G = int(n_graphs)
    N = x.shape[0]
    F = x.shape[1]
    FH = W1.shape[1]
    FO = Wfc.shape[1]
    assert F == FH, "kernel assumes F_IN == F_HID"

    # degrees (dst side, + self loop), as in the reference
    deg = np.bincount(ei[1], minlength=N).astype(np.float32) + 1.0
    dinv = 1.0 / np.sqrt(deg)
    sqdeg = np.sqrt(deg)

    # --- graph-padded node ordering (pure blocks) ---
    cnt = np.bincount(batch, minlength=G).astype(np.int64)  # nodes per graph
    blocks_g = _cdiv(cnt, P)  # 0 for empty graphs
    total_blocks = int(blocks_g.sum())
    total_blocks_padded = _cdiv(total_blocks, n_cores) * n_cores
    BPC = total_blocks_padded // n_cores
    RPC = BPC * P
    NP = total_blocks_padded * P
    HALF = NP // 2
    assert HALF <= 32768, f"table half {HALF} exceeds int16 index range"

    # AllGather chunking: 2 pieces, each one gather half (chunk == half), so
    # each chunk is a single Shared DRAM tile written by exactly one AG and
    # read as exactly one gather source region.
    NCHUNK = 2 if BPC % 2 == 0 else 1
    CHB = BPC // NCHUNK          # blocks per chunk (per core)
    CHR = CHB * P                # rows per chunk (per core)

    blk_start = np.concatenate([[0], np.cumsum(blocks_g)])  # per graph
    row_start = blk_start * P
    first_node = np.concatenate([[0], np.cumsum(cnt)])[:-1]

    def layout(block_perm=None):
        """node -> padded row, with optional per-core block permutation.

        block_perm[c][l] = pre-layout local block that lands at local slot l.
        """
        pre_pos = row_start[batch] + (np.arange(N) - first_node[batch])
        if block_perm is None:
            return pre_pos
        pre_blk = pre_pos // P
        c_of = pre_blk // BPC
        l_of = pre_blk % BPC
        # inverse: where does pre-local-block l of core c go?
        inv = np.zeros((n_cores, BPC), np.int64)
        for c in range(n_cores):
            inv[c, block_perm[c]] = np.arange(BPC)
        new_blk = c_of * BPC + inv[c_of, l_of]
        return new_blk * P + (pre_pos % P)

    # pass 1: preliminary layout to measure per-(core, block, half) counts
    new_pos = layout()
    es0, ed0 = new_pos[ei[0]], new_pos[ei[1]]
    cnt3p = np.zeros((n_cores, BPC, 2), np.int64)
    np.add.at(cnt3p, (ed0 // RPC, (ed0 % RPC) // P, es0 // HALF), 1)
    # sort blocks within each core by max-half count (desc) to align maxima
    key = np.maximum(cnt3p[:, :, 0], cnt3p[:, :, 1])
    block_perm = np.argsort(-key, axis=1)
    new_pos = layout(block_perm)

    row2node = np.full(NP, -1, np.int64)
    row2node[new_pos] = np.arange(N)
    real = row2node >= 0

    # per padded row data
    x_pad = np.zeros((NP, F), np.float32)
    x_pad[real] = x[row2node[real]]
    dinv_pad = np.ones(NP, np.float32)
    dinv_pad[real] = dinv[row2node[real]]
    sqdeg_pad = np.zeros(NP, np.float32)
    sqdeg_pad[real] = sqdeg[row2node[real]]
    xs_pad = (x_pad * dinv_pad[:, None]).astype(BF16)
    g_of_block = np.full(total_blocks_padded, -1, np.int64)
    gb = np.where(real, batch[np.clip(row2node, 0, N - 1)], -1)
    for j in range(total_blocks_padded):
        blkg = gb[j * P:(j + 1) * P]
        blkg = blkg[blkg >= 0]
        if blkg.size:
            g_of_block[j] = blkg[0]

    # --- edges WITHOUT self loops (self loops folded via identity matmul) ---
    es = new_pos[ei[0]]
    ed = new_pos[ei[1]]
    core = ed // RPC
    blk = (ed % RPC) // P
    slot = ed % P
    # gather-table address of a src row under the chunked AllGather layout:
    # chunk k holds [8 cores x CHR rows] at offset k*8*CHR.
    src_c = es // RPC
    src_lr = es % RPC
    src_k = src_lr // CHR
    gaddr = src_k * (n_cores * CHR) + src_c * CHR + (src_lr % CHR)
    half = gaddr // HALF
    lsrc = gaddr - half * HALF

    cnt3 = np.zeros((n_cores, BPC, 2), np.int64)
    np.add.at(cnt3, (core, blk, half), 1)
    T = np.max(_cdiv(cnt3, P), axis=0)  # [BPC, 2] tiles, uniform across cores

    # call / group structure: one gather call per (block, half) with tiles,
    # so per-core padding is trailing and skippable via num_idxs_reg.
    blocks_groups = [list(range(s, min(s + GROUP_NBLK, BPC)))
                     for s in range(0, BPC, GROUP_NBLK)]
    groups = []
    tt = 0
    idxcols = 0
    ncalls = 0
    tile_of = np.zeros((BPC, 2), np.int64)
    for gblocks in blocks_groups:
        calls = []
        g_t0 = tt
        for b in gblocks:
            for h in (0, 1):
                ntiles = int(T[b, h])
                if ntiles == 0:
                    continue
                tile_of[b, h] = tt
                calls.append(dict(b=b, h=h, ntiles=ntiles, tstart=tt,
                                  idx_off=idxcols, call_id=ncalls))
                tt += ntiles
                idxcols += ntiles * 8
                ncalls += 1
        groups.append(dict(blocks=gblocks, calls=calls,
                           tstart=g_t0, ntiles=tt - g_t0))
    TT = tt
    IDXCOLS = idxcols
    NCALLS = max(ncalls, 1)
    MAXG = max((g["ntiles"] for g in groups), default=1)

    # --- per-core edge arrays in tile order ---
    order = np.lexsort((lsrc, half, blk, core))
    so_lsrc, so_slot, so_src = lsrc[order], slot[order], es[order]
    run_start = np.zeros((n_cores, BPC, 2), np.int64)
    flat_cnt = cnt3.reshape(-1)
    np.cumsum(flat_cnt[:-1], out=run_start.reshape(-1)[1:])

    NOSKIP = bool(int(os.environ.get("V3_NOSKIP", "0")))
    idxflat = np.full((n_cores, TT * P), -1 if not NOSKIP else 0, np.int16)
    slotflat = np.full((n_cores, TT * P), 255.0, np.float32)
    srcflat = np.zeros((n_cores, TT * P), np.int64)
    validflat = np.zeros((n_cores, TT * P), bool)
    nvalid = np.ones((n_cores, NCALLS), np.int32)
    for c in range(n_cores):
        for b in range(BPC):
            for h in (0, 1):
                if T[b, h] == 0:
                    continue
                n = int(cnt3[c, b, h])
                s0 = int(run_start[c, b, h])
                o = int(tile_of[b, h]) * P
                if n == 0:
                    idxflat[c, o] = 0  # keep >=1 valid idx per call
                else:
                    idxflat[c, o:o + n] = so_lsrc[s0:s0 + n].astype(np.int16)
                    slotflat[c, o:o + n] = so_slot[s0:s0 + n]
                    srcflat[c, o:o + n] = so_src[s0:s0 + n]
                    validflat[c, o:o + n] = True
    for g in groups:
        for call in g["calls"]:
            b, h = call["b"], call["h"]
            if NOSKIP:
                nvalid[:, call["call_id"]] = call["ntiles"] * P
            else:
                nvalid[:, call["call_id"]] = np.maximum(cnt3[:, b, h], 1)

    # wrap-16 + replicate-to-128 index layout, call-local (layer 2 gather)
    gidx = np.zeros((n_cores, P, IDXCOLS), np.int16)
    for g in groups:
        for call in g["calls"]:
            a = call["tstart"] * P
            nt = call["ntiles"]
            region = idxflat[:, a:a + nt * P]           # [NC, nt*128]
            arr = region.reshape(n_cores, nt * 8, 16)   # i -> (i//16, i%16)
            arr = arr.transpose(0, 2, 1)                # [NC, 16, cols]
            gidx[:, :, call["idx_off"]:call["idx_off"] + nt * 8] = (
                np.tile(arr, (1, 8, 1)))
    gslot = slotflat.reshape(n_cores, TT, P).transpose(0, 2, 1).copy()

    # --- layer-1 pre-gathered message tiles (dinv_src * x[src]) ---
    xg = np.zeros((n_cores, P, TT * F), BF16)
    for c in range(n_cores):
        rows = xs_pad[srcflat[c]]                       # [TT*P, F] bf16
        rows[~validflat[c]] = 0
        xg[c] = rows.reshape(TT, P, F).transpose(1, 0, 2).reshape(P, TT * F)
    xself = np.zeros((n_cores, P, BPC * F), BF16)
    for c in range(n_cores):
        r0 = c * RPC
        xself[c] = (xs_pad[r0:r0 + RPC]
                    .reshape(BPC, P, F).transpose(1, 0, 2).reshape(P, BPC * F))

    # --- pooling helpers ---
    rows_i = np.arange(NP)
    rcore = rows_i // RPC
    rblk = (rows_i % RPC) // P
    rslot = rows_i % P
    pm = np.zeros((n_cores, P, BPC * G), BF16)
    pm[rcore[real], rslot[real], rblk[real] * G + gb[real]] = 1.0
    recip = (1.0 / np.maximum(cnt, 1.0)).astype(np.float32).reshape(G, 1)

    has_b1 = bool(np.any(np.asarray(b1)))
    has_b2 = bool(np.any(np.asarray(b2)))

    # --- per-core input maps ---
    in_maps = []
    for c in range(n_cores):
        r0, r1 = c * RPC, (c + 1) * RPC
        m = {
            "xg": xg[c],
            "xself": xself[c],
            "nvalid": nvalid[c].reshape(1, NCALLS),
            "w1": np.asarray(W1, np.float32).astype(BF16),
            "w2": np.asarray(W2, np.float32).astype(BF16),
            "wfc": np.asarray(Wfc, np.float32).astype(BF16),
            "b1r": np.asarray(b1, np.float32).reshape(1, FH).astype(BF16),
            "b2r": np.asarray(b2, np.float32).reshape(1, FH).astype(BF16),
            "bfcr": np.asarray(bfc, np.float32).reshape(1, FO).astype(BF16),
            "dinv": np.ascontiguousarray(
                dinv_pad[r0:r1].reshape(BPC, P).T).astype(np.float32),
            "gidx": gidx[c],
            "gslot": gslot[c],
            "pm": pm[c],
            "recip": recip,
        }
        if has_b1 or has_b2:
            m["sqdeg"] = sqdeg_pad[r0:r1].reshape(1, RPC).astype(BF16)
        in_maps.append(m)

    # graph id of each local block per core (host constant for the program;
    # same structure across cores is NOT required for data, but the program
    # needs a uniform instruction stream -> use per-core data via masks).
    # Incremental max uses g_of_block of THIS core; but the program must be
    # uniform, so instead we use a per-core "maxcol" input: column index in
    # mxT_loc for each block (or a dump column G for trash blocks).
    maxcol = np.zeros((n_cores, BPC), np.int64)
    for c in range(n_cores):
        for b in range(BPC):
            g = g_of_block[c * BPC + b]
            maxcol[c, b] = g if g >= 0 else G
    # maxcol differs per core -> cannot be baked into the (uniform) program.
    # Instead supply a per-core one-hot routing matrix per block is overkill;
    # we use a [P, BPC] bf16 "bsel" input: bsel[:, b] is all-ones if block b
    # is real, else zeros, and a per-core int map is impossible -- so we
    # instead accumulate per-BLOCK maxima into a [P, KC*BPC] buffer (uniform)
    # and do the masked per-graph reduction in the tail as before, but with
    # the mask multiply fused to KC*G vector ops over [P, BPC] tiles.
    pmask = np.zeros((n_cores, P, G * BPC), BF16)
    for c in range(n_cores):
        for b in range(BPC):
            g = g_of_block[c * BPC + b]
            if g >= 0:
                pmask[c, :, g * BPC + b] = 1.0
    for c in range(n_cores):
        in_maps[c]["pmask"] = pmask[c]

    plan = dict(
        G=G, F=F, FH=FH, FO=FO, BPC=BPC, RPC=RPC, NP=NP, HALF=HALF,
        TT=TT, IDXCOLS=IDXCOLS, NCALLS=NCALLS, MAXG=MAXG, groups=groups,
        NCHUNK=NCHUNK, CHB=CHB,
        n_cores=n_cores,
        has_b1=has_b1,
        has_b2=has_b2,
        has_bfc=bool(np.any(np.asarray(bfc))),
        MAXCT=MAXG,  # for test harness prints
    )
    return plan, in_maps


# --------------------------------------------------------------------------
# Bass program builder (identical on all cores).
# --------------------------------------------------------------------------

def build(plan, debug=False):
    dt = mybir.dt
    G, F, FH, FO = plan["G"], plan["F"], plan["FH"], plan["FO"]
    BPC, RPC, NP, HALF = plan["BPC"], plan["RPC"], plan["NP"], plan["HALF"]
    TT, IDXCOLS, NCALLS = plan["TT"], plan["IDXCOLS"], plan["NCALLS"]
    MAXG = plan["MAXG"]
    NCHUNK, CHB = plan["NCHUNK"], plan["CHB"]
    groups = plan["groups"]
    n_cores = plan["n_cores"]
    KC = F // P  # k-chunks for the transforms (2)
    FCK = (3 * FH) // P  # k-chunks for the FC (6)
    has_bias = plan["has_b1"] or plan["has_b2"]
    SP = bool(int(os.environ.get("SP", "0")))
    NOVL = bool(int(os.environ.get("V3_NOVL", "0")))     # no value_load
    NOWARM = bool(int(os.environ.get("V3_NOWARM", "0")))  # no warmup AR
    VECOH = bool(int(os.environ.get("V3_VECOH", "0")))    # all one-hots on DVE

    nc = bacc.Bacc("TRN2", target_bir_lowering=False, debug=debug,
                   num_devices=n_cores)

    def din(name, shape, dtype):
        return nc.dram_tensor(name, shape, dtype, kind="ExternalInput").ap()

    xg_d = din("xg", [P, TT * F], dt.bfloat16)
    xself_d = din("xself", [P, BPC * F], dt.bfloat16)
    nvalid_d = din("nvalid", [1, NCALLS], dt.int32)
    w1_d = din("w1", [F, FH], dt.bfloat16)
    w2_d = din("w2", [FH, FH], dt.bfloat16)
    wfc_d = din("wfc", [3 * FH, FO], dt.bfloat16)
    b1r_d = din("b1r", [1, FH], dt.bfloat16)
    b2r_d = din("b2r", [1, FH], dt.bfloat16)
    bfcr_d = din("bfcr", [1, FO], dt.bfloat16)
    if has_bias:
        sqdeg_d = din("sqdeg", [1, RPC], dt.bfloat16)
    dinv_d = din("dinv", [P, BPC], dt.float32)
    gidx_d = din("gidx", [P, IDXCOLS], dt.int16)
    gslot_d = din("gslot", [P, TT], dt.float32)
    pm_d = din("pm", [P, BPC * G], dt.bfloat16)
    pmask_d = din("pmask", [P, G * BPC], dt.bfloat16)
    recip_d = din("recip", [G, 1], dt.float32)
    out_d = nc.dram_tensor("out", [G, FO], dt.float32,
                           kind="ExternalOutput").ap()

    rg = [list(range(n_cores))]

    from contextlib import ExitStack
    with tile.TileContext(nc) as tc, ExitStack() as ctx:
        const = ctx.enter_context(tc.tile_pool(name="const", bufs=1))
        dram = ctx.enter_context(tc.tile_pool(name="dram", bufs=1, space="DRAM"))
        tfpsum = ctx.enter_context(tc.tile_pool(name="tfpsum", bufs=2, space="PSUM"))
        aggpsum = ctx.enter_context(tc.tile_pool(name="aggpsum", bufs=2, space="PSUM"))
        tpsum = ctx.enter_context(tc.tile_pool(name="tpsum", bufs=2, space="PSUM"))
        spsum = ctx.enter_context(tc.tile_pool(name="spsum", bufs=1, space="PSUM"))
        fcpsum = ctx.enter_context(tc.tile_pool(name="fcpsum", bufs=1, space="PSUM"))
        msgp = ctx.enter_context(tc.tile_pool(name="msgp", bufs=2))
        btp = ctx.enter_context(tc.tile_pool(name="btp", bufs=8))
        hp = ctx.enter_context(tc.tile_pool(name="hp", bufs=4))
        htp = ctx.enter_context(tc.tile_pool(name="htp", bufs=6))
        tailp = ctx.enter_context(tc.tile_pool(name="tailp", bufs=1))

        # ---------------- constants into SBUF ----------------
        def cload(tag, dram_ap, shape, dtype):
            t = const.tile(shape, dtype, tag=tag)
            nc.sync.dma_start(out=t[:], in_=dram_ap)
            return t

        w_sb = []
        for tag, d in (("w1", w1_d), ("w2", w2_d)):
            t = const.tile([P, KC * FH], dt.bfloat16, tag=tag)
            for c in range(KC):
                nc.sync.dma_start(out=t[:, c * FH:(c + 1) * FH],
                                  in_=d[c * P:(c + 1) * P, :])
            w_sb.append(t)
        wfc_sb = const.tile([P, FCK * FO], dt.bfloat16, tag="wfc")
        for c in range(FCK):
            nc.sync.dma_start(out=wfc_sb[:, c * FO:(c + 1) * FO],
                              in_=wfc_d[c * P:(c + 1) * P, :])
        b1r_sb = cload("b1r", b1r_d, [1, FH], dt.bfloat16)
        b2r_sb = cload("b2r", b2r_d, [1, FH], dt.bfloat16)
        bfcr_sb = cload("bfcr", bfcr_d, [1, FO], dt.bfloat16)
        if has_bias:
            sqdeg_sb = cload("sqdeg", sqdeg_d, [1, RPC], dt.bfloat16)
        dinv_sb = cload("dinv", dinv_d, [P, BPC], dt.float32)
        gidx_sb = cload("gidx", gidx_d, [P, IDXCOLS], dt.int16)
        gslot_sb = cload("gslot", gslot_d, [P, TT], dt.float32)
        nvalid_sb = cload("nvalid", nvalid_d, [1, NCALLS], dt.int32)
        xself_sb = cload("xself", xself_d, [P, BPC * F], dt.bfloat16)
        pm_sb = cload("pm", pm_d, [P, BPC * G], dt.bfloat16)
        pmask_sb = cload("pmask", pmask_d, [P, G * BPC], dt.bfloat16)
        recip_sb = cload("recip", recip_d, [G, 1], dt.float32)

        iota_sb = const.tile([P, P], dt.float32, tag="iota")
        nc.gpsimd.iota(out=iota_sb[:], pattern=[[1, P]], base=0,
                       channel_multiplier=0,
                       allow_small_or_imprecise_dtypes=True)
        iotac_sb = const.tile([P, 1], dt.float32, tag="iotac")
        nc.gpsimd.iota(out=iotac_sb[:], pattern=[[0, 1]], base=0,
                       channel_multiplier=1,
                       allow_small_or_imprecise_dtypes=True)
        ident_sb = const.tile([P, P], dt.bfloat16, tag="ident")
        nc.vector.tensor_tensor(out=ident_sb[:],
                                in0=iotac_sb[:].to_broadcast([P, P]),
                                in1=iota_sb[:],
                                op=mybir.AluOpType.is_equal)
        ones_sb = const.tile([1, G], dt.bfloat16, tag="ones")
        nc.gpsimd.memset(ones_sb[:], 1.0)
        tbl2own = const.tile([P, BPC * FH], dt.bfloat16, tag="tbl2own")
        blockmax = const.tile([P, KC * BPC], dt.bfloat16, tag="bmax")

        # DRAM bounce buffers for collectives
        ag_in = dram.tile([RPC, FH], dt.bfloat16, name="agin", tag="agin")
        ag_outs = [dram.tile([n_cores * CHB * P, FH], dt.bfloat16,
                             name=f"agout{k}", tag=f"agout{k}",
                             addr_space="Shared")
                   for k in range(NCHUNK)]
        ars_in = dram.tile([G, FH], dt.float32, tag="arsin")
        ars_out = dram.tile([G, FH], dt.float32, tag="arsout",
                            addr_space="Shared")
        arm_in = dram.tile([P, KC * G], dt.bfloat16, tag="armin")
        arm_out = dram.tile([P, KC * G], dt.bfloat16, tag="armout",
                            addr_space="Shared")
        warm_in = dram.tile([1, 16], dt.float32, tag="warmin")
        warm_out = dram.tile([1, 16], dt.float32, tag="warmout",
                             addr_space="Shared")

        Copy = mybir.ActivationFunctionType.Copy
        Relu = mybir.ActivationFunctionType.Relu

        # warm up the collectives stack (entry barrier etc.) during layer 1
        if not NOWARM:
            warm_sb = tailp.tile([1, 16], dt.float32, tag="warm_sb")
            nc.gpsimd.memset(warm_sb[:], 0.0)
            nc.sync.dma_start(out=warm_in[:], in_=warm_sb[:])
            nc.gpsimd.collective_compute(
                "AllReduce", mybir.AluOpType.add,
                ins=[warm_in[:].opt()], outs=[warm_out[:].opt()],
                replica_groups=rg)

        def build_onehot(gt, eng):
            bt = btp.tile([P, P], dt.bfloat16, tag="bt")
            eng.tensor_scalar(
                out=bt[:], in0=iota_sb[:],
                scalar1=gslot_sb[:, gt:gt + 1], scalar2=None,
                op0=mybir.AluOpType.is_equal)
            return bt

        # ---------------- layer 1: scatter pre-gathered X, then transform --
        done_blocks = 0
        next_chunk = 0
        for grp in groups:
            nt_g = grp["ntiles"]
            if nt_g > 0:
                xgt = msgp.tile([P, MAXG * F], dt.bfloat16, tag="msg")
                a = grp["tstart"] * F
                nc.sync.dma_start(out=xgt[:, :nt_g * F],
                                  in_=xg_d[:, a:a + nt_g * F])
            for b in grp["blocks"]:
                calls_b = [c for c in grp["calls"] if c["b"] == b]
                nmm = sum(c["ntiles"] for c in calls_b)
                ps = aggpsum.tile([P, FH], dt.float32, tag="aggps")
                nc.tensor.matmul(
                    out=ps[:], lhsT=ident_sb[:],
                    rhs=xself_sb[:, b * F:(b + 1) * F],
                    start=True,
                    stop=(nmm == 0 and not plan["has_b1"]))
                k = 0
                for call in calls_b:
                    for t in range(call["ntiles"]):
                        gt = call["tstart"] + t
                        loc = gt - grp["tstart"]
                        bt = build_onehot(gt, nc.vector if VECOH
                                          else nc.gpsimd)
                        k += 1
                        nc.tensor.matmul(
                            out=ps[:], lhsT=bt[:],
                            rhs=xgt[:, loc * F:(loc + 1) * F],
                            start=False,
                            stop=(k == nmm) and not plan["has_b1"])
                if plan["has_b1"]:
                    nc.tensor.matmul(
                        out=ps[:],
                        lhsT=sqdeg_sb[:, b * P:(b + 1) * P],
                        rhs=b1r_sb[:],
                        start=False, stop=True)

                # epilogue: t1 = dinv*S; h1 = relu(t1 @ W1);
                # table2 = dinv * (h1 @ W2)
                t1 = hp.tile([P, FH], dt.bfloat16, tag="t1")
                nc.scalar.activation(out=t1[:], in_=ps[:], func=Copy,
                                     scale=dinv_sb[:, b:b + 1])
                ps2 = tfpsum.tile([P, FH], dt.float32, tag="tfps")
                for c in range(KC):
                    tp = tpsum.tile([P, P], dt.bfloat16, tag="tp")
                    nc.tensor.transpose(out=tp[:],
                                        in_=t1[:, c * P:(c + 1) * P],
                                        identity=ident_sb[:])
                    ht = htp.tile([P, P], dt.bfloat16, tag="ht")
                    nc.vector.tensor_copy(out=ht[:], in_=tp[:])
                    nc.tensor.matmul(out=ps2[:], lhsT=ht[:],
                                     rhs=w_sb[0][:, c * FH:(c + 1) * FH],
                                     start=(c == 0), stop=(c == KC - 1))
                h1 = hp.tile([P, FH], dt.bfloat16, tag="h1")
                nc.scalar.activation(out=h1[:], in_=ps2[:], func=Relu)
                ps3 = tfpsum.tile([P, FH], dt.float32, tag="tfps")
                for c in range(KC):
                    tp = tpsum.tile([P, P], dt.bfloat16, tag="tp")
                    nc.tensor.transpose(out=tp[:],
                                        in_=h1[:, c * P:(c + 1) * P],
                                        identity=ident_sb[:])
                    ht = htp.tile([P, P], dt.bfloat16, tag="ht")
                    nc.vector.tensor_copy(out=ht[:], in_=tp[:])
                    nc.tensor.matmul(out=ps3[:], lhsT=ht[:],
                                     rhs=w_sb[1][:, c * FH:(c + 1) * FH],
                                     start=(c == 0), stop=(c == KC - 1))
                nc.scalar.activation(out=tbl2own[:, b * FH:(b + 1) * FH],
                                     in_=ps3[:], func=Copy,
                                     scale=dinv_sb[:, b:b + 1])
                nc.sync.dma_start(out=ag_in[b * P:(b + 1) * P, :],
                                  in_=tbl2own[:, b * FH:(b + 1) * FH])
                done_blocks += 1
                # fire AllGather chunks as soon as their blocks are written
                while (next_chunk < NCHUNK
                       and done_blocks >= (next_chunk + 1) * CHB):
                    r0 = next_chunk * CHB * P
                    r1 = (next_chunk + 1) * CHB * P
                    nc.gpsimd.collective_compute(
                        "AllGather", mybir.AluOpType.bypass,
                        ins=[ag_in[r0:r1, :].opt()],
                        outs=[ag_outs[next_chunk][:].opt()],
                        replica_groups=rg)
                    next_chunk += 1

        # ---------------- layer 2: gather + scatter + pooling epilogue ----
        sums_ps = spsum.tile([G, FH], dt.float32, tag="sums")

        for grp in groups:
            mb = None
            nt_g = grp["ntiles"]
            if nt_g > 0:
                mb = msgp.tile([P, MAXG * FH], dt.bfloat16, tag="msg")
                # The gather skips per-core trailing padding (negative idxs);
                # zero the buffer first so skipped rows read as 0 (their
                # one-hot columns are zero, but 0 * NaN would poison PSUM).
                nc.vector.memset(mb[:, :nt_g * FH], 0.0)
                for call in grp["calls"]:
                    h, nt = call["h"], call["ntiles"]
                    loc0 = call["tstart"] - grp["tstart"]
                    out_ap = mb[:, loc0 * FH:(loc0 + nt) * FH].rearrange(
                        "p (t e) -> p t e", e=FH)
                    if NOVL:
                        nv = nt * P
                    else:
                        # NOTE: min_val/max_val would emit a runtime-assert
                        # sequencer instruction that hangs on HW.
                        nv = nc.gpsimd.value_load(
                            nvalid_sb[0:1,
                                      call["call_id"]:call["call_id"] + 1])
                    table_ap = (ag_outs[h][:] if NCHUNK == 2
                                else ag_outs[0][h * HALF:(h + 1) * HALF, :])
                    nc.gpsimd.dma_gather(
                        out_ap=out_ap,
                        in_ap=table_ap,
                        idxs_ap=gidx_sb[:, call["idx_off"]:
                                        call["idx_off"] + nt * 8],
                        num_idxs=nt * P,
                        num_idxs_reg=nv,
                        elem_size=FH,
                        single_packet=SP)
            for b in grp["blocks"]:
                calls_b = [c for c in grp["calls"] if c["b"] == b]
                nmm = sum(c["ntiles"] for c in calls_b)
                ps = aggpsum.tile([P, FH], dt.float32, tag="aggps")
                nc.tensor.matmul(
                    out=ps[:], lhsT=ident_sb[:],
                    rhs=tbl2own[:, b * FH:(b + 1) * FH],
                    start=True,
                    stop=(nmm == 0 and not plan["has_b2"]))
                k = 0
                for call in calls_b:
                    loc0 = call["tstart"] - grp["tstart"]
                    for t in range(call["ntiles"]):
                        gt = call["tstart"] + t
                        bt = build_onehot(gt, nc.vector)
                        k += 1
                        nc.tensor.matmul(
                            out=ps[:], lhsT=bt[:],
                            rhs=mb[:, (loc0 + t) * FH:(loc0 + t + 1) * FH],
                            start=False,
                            stop=(k == nmm) and not plan["has_b2"])
                if plan["has_b2"]:
                    nc.tensor.matmul(
                        out=ps[:],
                        lhsT=sqdeg_sb[:, b * P:(b + 1) * P],
                        rhs=b2r_sb[:],
                        start=False, stop=True)
                # epilogue: h2 = relu(dinv * ps); pooling contributions
                h2 = hp.tile([P, FH], dt.bfloat16, tag="h2")
                nc.scalar.activation(out=h2[:], in_=ps[:], func=Relu,
                                     scale=dinv_sb[:, b:b + 1])
                nc.tensor.matmul(out=sums_ps[:],
                                 lhsT=pm_sb[:, b * G:(b + 1) * G],
                                 rhs=h2[:],
                                 start=(b == 0), stop=(b == BPC - 1))
                for c in range(KC):
                    tp = tpsum.tile([P, P], dt.bfloat16, tag="tp")
                    nc.tensor.transpose(out=tp[:],
                                        in_=h2[:, c * P:(c + 1) * P],
                                        identity=ident_sb[:])
                    nc.vector.tensor_reduce(
                        out=blockmax[:, c * BPC + b:c * BPC + b + 1],
                        in_=tp[:], axis=mybir.AxisListType.X,
                        op=mybir.AluOpType.max)

        # ---------------- pooling tail ----------------
        sums_sb = tailp.tile([G, FH], dt.float32, tag="sums_sb")
        nc.vector.tensor_copy(out=sums_sb[:], in_=sums_ps[:])
        nc.sync.dma_start(out=ars_in[:], in_=sums_sb[:])
        nc.gpsimd.collective_compute(
            "AllReduce", mybir.AluOpType.add,
            ins=[ars_in[:].opt()], outs=[ars_out[:].opt()],
            replica_groups=rg)
        # per-graph LOCAL max from this core's block maxima via masks
        mxT_loc = tailp.tile([P, KC * G], dt.bfloat16, tag="mxT_loc")
        mtmp = tailp.tile([P, BPC], dt.bfloat16, tag="mtmp")
        for g in range(G):
            for c in range(KC):
                nc.vector.tensor_tensor(
                    out=mtmp[:], in0=blockmax[:, c * BPC:(c + 1) * BPC],
                    in1=pmask_sb[:, g * BPC:(g + 1) * BPC],
                    op=mybir.AluOpType.mult)
                nc.vector.tensor_reduce(
                    out=mxT_loc[:, c * G + g:c * G + g + 1], in_=mtmp[:],
                    axis=mybir.AxisListType.X, op=mybir.AluOpType.max)
        nc.sync.dma_start(out=arm_in[:], in_=mxT_loc[:])
        nc.gpsimd.collective_compute(
            "AllReduce", mybir.AluOpType.max,
            ins=[arm_in[:].opt()], outs=[arm_out[:].opt()],
            replica_groups=rg)

        gsums = tailp.tile([G, FH], dt.float32, tag="gsums")
        nc.sync.dma_start(out=gsums[:], in_=ars_out[:])
        mxT = tailp.tile([P, KC * G], dt.bfloat16, tag="mxT")
        nc.sync.dma_start(out=mxT[:], in_=arm_out[:])

        # mean / sums in bf16, transposed to feature-major for the FC
        mean_sb = tailp.tile([G, FH], dt.bfloat16, tag="mean")
        nc.vector.tensor_scalar(out=mean_sb[:], in0=gsums[:],
                                scalar1=recip_sb[:], scalar2=None,
                                op0=mybir.AluOpType.mult)
        sums_bf = tailp.tile([G, FH], dt.bfloat16, tag="sumsbf")
        nc.vector.tensor_copy(out=sums_bf[:], in_=gsums[:])
        meanT = tailp.tile([P, KC * G], dt.bfloat16, tag="meanT")
        sumsT = tailp.tile([P, KC * G], dt.bfloat16, tag="sumsT")
        for src, dst_t in ((mean_sb, meanT), (sums_bf, sumsT)):
            for c in range(KC):
                tp = tpsum.tile([P, P], dt.bfloat16, tag="tp")
                nc.tensor.transpose(out=tp[:, :G],
                                    in_=src[:, c * P:(c + 1) * P],
                                    identity=ident_sb[:G, :G])
                nc.vector.tensor_copy(out=dst_t[:, c * G:(c + 1) * G],
                                      in_=tp[:, :G])

        # final FC: out = [mean | max | sums] @ Wfc + bfc
        fc_ps = fcpsum.tile([G, FO], dt.float32, tag="fc")
        gT = [meanT, mxT, sumsT]
        k = 0
        for part in range(3):
            for c in range(KC):
                nc.tensor.matmul(
                    out=fc_ps[:], lhsT=gT[part][:, c * G:(c + 1) * G],
                    rhs=wfc_sb[:, k * FO:(k + 1) * FO],
                    start=(k == 0),
                    stop=(k == FCK - 1) and not plan["has_bfc"])
                k += 1
        if plan["has_bfc"]:
            nc.tensor.matmul(out=fc_ps[:], lhsT=ones_sb[:], rhs=bfcr_sb[:],
                             start=False, stop=True)
        out_sb = tailp.tile([G, FO], dt.float32, tag="out_sb")
        nc.vector.tensor_copy(out=out_sb[:], in_=fc_ps[:])
        nc.sync.dma_start(out=out_d[:], in_=out_sb[:])

    nc.compile()
    return nc


# --------------------------------------------------------------------------
# Entry point for the grading harness.
# --------------------------------------------------------------------------

def kernel(x, edge_index, batch, n_graphs, W1, b1, W2, b2, Wfc, bfc,
           **_unused):
    plan, in_maps = preprocess(x, edge_index, batch, n_graphs,
                               W1, b1, W2, b2, Wfc, bfc)
    nc = build(plan)
    res = run_bass_kernel_spmd(nc, in_maps, core_ids=list(range(NCORES)))
    out = np.asarray(res.results[0]["out"], np.float32)
    return out


# revision 30
# speedup vs baseline: 2.2419x; 2.2419x over previous
"""Trainium2 Bass kernel for a 2-layer GCN (EnhancedGNN) with triple global
pooling and a final FC, run SPMD across 8 NeuronCores.

Strategy v3:
  - Nodes are re-ordered into graph-pure 128-row blocks, padded per graph,
    sharded contiguously across the 8 cores (dst / data parallel). Within
    each core, blocks are sorted by edge count so the max-over-cores tile
    padding shrinks (the SPMD program uses per-(block,half) maxima).
  - Layer 1 does NOT gather on device: since X is a replicated input and
    scatter/transform commute ((B^T Xg) @ W1 == B^T (Xg @ W1)), the host
    pre-lays-out dinv-scaled X rows in message-tile order and the device
    streams them with affine DMA, scatters them into per-dst-block PSUM via
    one-hot matmuls, then applies W1 (and W2 to build the layer-2 table).
  - Self-loop messages fold in as an identity matmul against the block's
    own rows (X rows for layer 1, own layer-2 table rows for layer 2).
  - The layer-2 table AllGather is chunked (NCHUNK pieces) so all but the
    last chunk overlap layer-1 compute; a tiny warmup AllReduce at program
    start absorbs the collectives' entry barrier.
  - Layer 2: per-(block,half) dma_gather calls with trailing -1 padding and
    a per-core runtime num_idxs_reg, so the GpSimd Q7 descriptor generation
    (the measured bottleneck) only pays for real edges on each core.
  - Pooling: per-graph sums via one-hot matmul; per-graph max accumulated
    incrementally per block (vector max) during layer 2; AllReduce(add/max);
    the tiny FC runs redundantly on every core.

The kernel program is identical on all 8 cores (SPMD); all per-core
differences live in the input data. Structure constants (tile counts etc.)
are maxima over cores so the program is uniform.
"""

import os
import numpy as np
import ml_dtypes

import concourse.bass as bass
import concourse.tile as tile
from concourse import bacc, mybir
from concourse.bass_utils import run_bass_kernel_spmd

P = 128
NCORES = 8
GROUP_NBLK = 2  # dst blocks per gather/stream group

BF16 = ml_dtypes.bfloat16


def _cdiv(a, b):
    return -(-a // b)


# --------------------------------------------------------------------------
# Host-side preprocessing: sharding, edge grouping, auxiliary tensors.
# --------------------------------------------------------------------------

def preprocess(x, edge_index, batch, n_graphs, W1, b1, W2, b2, Wfc, bfc,
               n_cores=NCORES):
    x = np.asarray(x, np.float32)
    ei = np.asarray(edge_index, np.int64)
    batch = np.asarray(batch, np.int64)
    G = int(n_graphs)
    N = x.shape[0]
    F = x.shape[1]
    FH = W1.shape[1]
    FO = Wfc.shape[1]
    assert F == FH, "kernel assumes F_IN == F_HID"

    # degrees (dst side, + self loop), as in the reference
    deg = np.bincount(ei[1], minlength=N).astype(np.float32) + 1.0
    dinv = 1.0 / np.sqrt(deg)
    sqdeg = np.sqrt(deg)

    # --- graph-padded node ordering (pure blocks) ---
    cnt = np.bincount(batch, minlength=G).astype(np.int64)  # nodes per graph
    blocks_g = _cdiv(cnt, P)  # 0 for empty graphs
    total_blocks = int(blocks_g.sum())
    total_blocks_padded = _cdiv(total_blocks, n_cores) * n_cores
    BPC = total_blocks_padded // n_cores
    RPC = BPC * P
    NP = total_blocks_padded * P
    HALF = NP // 2
    assert HALF <= 32768, f"table half {HALF} exceeds int16 index range"

    # AllGather chunking: 2 pieces, each one gather half (chunk == half), so
    # each chunk is a single Shared DRAM tile written by exactly one AG and
    # read as exactly one gather source region.
    NCHUNK = 2 if BPC % 2 == 0 else 1
    CHB = BPC // NCHUNK          # blocks per chunk (per core)
    CHR = CHB * P                # rows per chunk (per core)

    blk_start = np.concatenate([[0], np.cumsum(blocks_g)])  # per graph
    row_start = blk_start * P
    first_node = np.concatenate([[0], np.cumsum(cnt)])[:-1]

    def layout(block_perm=None):
        """node -> padded row, with optional per-core block permutation.

        block_perm[c][l] = pre-layout local block that lands at local slot l.
        """
        pre_pos = row_start[batch] + (np.arange(N) - first_node[batch])
        if block_perm is None:
            return pre_pos
        pre_blk = pre_pos // P
        c_of = pre_blk // BPC
        l_of = pre_blk % BPC
        # inverse: where does pre-local-block l of core c go?
        inv = np.zeros((n_cores, BPC), np.int64)
        for c in range(n_cores):
            inv[c, block_perm[c]] = np.arange(BPC)
        new_blk = c_of * BPC + inv[c_of, l_of]
        return new_blk * P + (pre_pos % P)

    # pass 1: preliminary layout to measure per-(core, block, half) counts
    new_pos = layout()
    es0, ed0 = new_pos[ei[0]], new_pos[ei[1]]
    cnt3p = np.zeros((n_cores, BPC, 2), np.int64)
    np.add.at(cnt3p, (ed0 // RPC, (ed0 % RPC) // P, es0 // HALF), 1)
    # sort blocks within each core by max-half count (desc) to align maxima
    key = np.maximum(cnt3p[:, :, 0], cnt3p[:, :, 1])
    block_perm = np.argsort(-key, axis=1)
    new_pos = layout(block_perm)

    row2node = np.full(NP, -1, np.int64)
    row2node[new_pos] = np.arange(N)
    real = row2node >= 0

    # per padded row data
    x_pad = np.zeros((NP, F), np.float32)
    x_pad[real] = x[row2node[real]]
    dinv_pad = np.ones(NP, np.float32)
    dinv_pad[real] = dinv[row2node[real]]
    sqdeg_pad = np.zeros(NP, np.float32)
    sqdeg_pad[real] = sqdeg[row2node[real]]
    xs_pad = (x_pad * dinv_pad[:, None]).astype(BF16)
    g_of_block = np.full(total_blocks_padded, -1, np.int64)
    gb = np.where(real, batch[np.clip(row2node, 0, N - 1)], -1)
    for j in range(total_blocks_padded):
        blkg = gb[j * P:(j + 1) * P]
        blkg = blkg[blkg >= 0]
        if blkg.size:
            g_of_block[j] = blkg[0]

    # --- edges WITHOUT self loops (self loops folded via identity matmul) ---
    es = new_pos[ei[0]]
    ed = new_pos[ei[1]]
    core = ed // RPC
    blk = (ed % RPC) // P
    slot = ed % P
    # gather-table address of a src row under the chunked AllGather layout:
    # chunk k holds [8 cores x CHR rows] at offset k*8*CHR.
    src_c = es // RPC
    src_lr = es % RPC
    src_k = src_lr // CHR
    gaddr = src_k * (n_cores * CHR) + src_c * CHR + (src_lr % CHR)
    half = gaddr // HALF
    lsrc = gaddr - half * HALF

    cnt3 = np.zeros((n_cores, BPC, 2), np.int64)
    np.add.at(cnt3, (core, blk, half), 1)
    T = np.max(_cdiv(cnt3, P), axis=0)  # [BPC, 2] tiles, uniform across cores

    # call / group structure: one gather call per (group, half) spanning the
    # group's blocks — few large calls amortize the ~µs fixed cost per call.
    blocks_groups = [list(range(s, min(s + GROUP_NBLK, BPC)))
                     for s in range(0, BPC, GROUP_NBLK)]
    groups = []
    tt = 0
    idxcols = 0
    ncalls = 0
    tile_of = np.zeros((BPC, 2), np.int64)
    for gblocks in blocks_groups:
        calls = []
        g_t0 = tt
        for h in (0, 1):
            ntiles = int(sum(T[b, h] for b in gblocks))
            if ntiles == 0:
                continue
            blocks_in_call = []
            t0 = 0
            for b in gblocks:
                tile_of[b, h] = tt + t0
                blocks_in_call.append((b, t0, int(T[b, h])))
                t0 += int(T[b, h])
            calls.append(dict(h=h, ntiles=ntiles, tstart=tt,
                              idx_off=idxcols, call_id=ncalls,
                              blocks=blocks_in_call))
            tt += ntiles
            idxcols += ntiles * 8
            ncalls += 1
        groups.append(dict(blocks=gblocks, calls=calls,
                           tstart=g_t0, ntiles=tt - g_t0))
    TT = tt
    IDXCOLS = idxcols
    NCALLS = max(ncalls, 1)
    MAXG = max((g["ntiles"] for g in groups), default=1)

    # --- per-core edge arrays in tile order ---
    order = np.lexsort((lsrc, half, blk, core))
    so_lsrc, so_slot, so_src = lsrc[order], slot[order], es[order]
    run_start = np.zeros((n_cores, BPC, 2), np.int64)
    flat_cnt = cnt3.reshape(-1)
    np.cumsum(flat_cnt[:-1], out=run_start.reshape(-1)[1:])

    idxflat = np.zeros((n_cores, TT * P), np.int16)
    slotflat = np.full((n_cores, TT * P), 255.0, np.float32)
    srcflat = np.zeros((n_cores, TT * P), np.int64)
    validflat = np.zeros((n_cores, TT * P), bool)
    for c in range(n_cores):
        for b in range(BPC):
            for h in (0, 1):
                if T[b, h] == 0:
                    continue
                n = int(cnt3[c, b, h])
                if n == 0:
                    continue
                s0 = int(run_start[c, b, h])
                o = int(tile_of[b, h]) * P
                idxflat[c, o:o + n] = so_lsrc[s0:s0 + n].astype(np.int16)
                slotflat[c, o:o + n] = so_slot[s0:s0 + n]
                srcflat[c, o:o + n] = so_src[s0:s0 + n]
                validflat[c, o:o + n] = True

    # wrap-16 + replicate-to-128 index layout, call-local (layer 2 gather)
    gidx = np.zeros((n_cores, P, IDXCOLS), np.int16)
    for g in groups:
        for call in g["calls"]:
            a = call["tstart"] * P
            nt = call["ntiles"]
            region = idxflat[:, a:a + nt * P]           # [NC, nt*128]
            arr = region.reshape(n_cores, nt * 8, 16)   # i -> (i//16, i%16)
            arr = arr.transpose(0, 2, 1)                # [NC, 16, cols]
            gidx[:, :, call["idx_off"]:call["idx_off"] + nt * 8] = (
                np.tile(arr, (1, 8, 1)))
    gslot = slotflat.reshape(n_cores, TT, P).transpose(0, 2, 1).copy()

    # --- layer-1 pre-gathered message tiles (dinv_src * x[src]) ---
    xg = np.zeros((n_cores, P, TT * F), BF16)
    for c in range(n_cores):
        rows = xs_pad[srcflat[c]]                       # [TT*P, F] bf16
        rows[~validflat[c]] = 0
        xg[c] = rows.reshape(TT, P, F).transpose(1, 0, 2).reshape(P, TT * F)
    xself = np.zeros((n_cores, P, BPC * F), BF16)
    for c in range(n_cores):
        r0 = c * RPC
        xself[c] = (xs_pad[r0:r0 + RPC]
                    .reshape(BPC, P, F).transpose(1, 0, 2).reshape(P, BPC * F))

    # --- pooling helpers ---
    rows_i = np.arange(NP)
    rcore = rows_i // RPC
    rblk = (rows_i % RPC) // P
    rslot = rows_i % P
    pm = np.zeros((n_cores, P, BPC * G), BF16)
    pm[rcore[real], rslot[real], rblk[real] * G + gb[real]] = 1.0
    recip = (1.0 / np.maximum(cnt, 1.0)).astype(np.float32).reshape(G, 1)

    has_b1 = bool(np.any(np.asarray(b1)))
    has_b2 = bool(np.any(np.asarray(b2)))

    # --- per-core input maps ---
    in_maps = []
    for c in range(n_cores):
        r0, r1 = c * RPC, (c + 1) * RPC
        m = {
            "xg": xg[c],
            "xself": xself[c],
            "w1": np.asarray(W1, np.float32).astype(BF16),
            "w2": np.asarray(W2, np.float32).astype(BF16),
            "wfc": np.asarray(Wfc, np.float32).astype(BF16),
            "b1r": np.asarray(b1, np.float32).reshape(1, FH).astype(BF16),
            "b2r": np.asarray(b2, np.float32).reshape(1, FH).astype(BF16),
            "bfcr": np.asarray(bfc, np.float32).reshape(1, FO).astype(BF16),
            "dinv": np.ascontiguousarray(
                dinv_pad[r0:r1].reshape(BPC, P).T).astype(np.float32),
            "gidx": gidx[c],
            "gslot": gslot[c],
            "pm": pm[c],
            "recip": recip,
        }
        if has_b1 or has_b2:
            m["sqdeg"] = sqdeg_pad[r0:r1].reshape(1, RPC).astype(BF16)
        in_maps.append(m)

    # graph id of each local block per core (host constant for the program;
    # same structure across cores is NOT required for data, but the program
    # needs a uniform instruction stream -> use per-core data via masks).
    # Incremental max uses g_of_block of THIS core; but the program must be
    # uniform, so instead we use a per-core "maxcol" input: column index in
    # mxT_loc for each block (or a dump column G for trash blocks).
    maxcol = np.zeros((n_cores, BPC), np.int64)
    for c in range(n_cores):
        for b in range(BPC):
            g = g_of_block[c * BPC + b]
            maxcol[c, b] = g if g >= 0 else G
    # maxcol differs per core -> cannot be baked into the (uniform) program.
    # Instead supply a per-core one-hot routing matrix per block is overkill;
    # we use a [P, BPC] bf16 "bsel" input: bsel[:, b] is all-ones if block b
    # is real, else zeros, and a per-core int map is impossible -- so we
    # instead accumulate per-BLOCK maxima into a [P, KC*BPC] buffer (uniform)
    # and do the masked per-graph reduction in the tail as before, but with
    # the mask multiply fused to KC*G vector ops over [P, BPC] tiles.
    pmask = np.zeros((n_cores, P, G * BPC), BF16)
    for c in range(n_cores):
        for b in range(BPC):
            g = g_of_block[c * BPC + b]
            if g >= 0:
                pmask[c, :, g * BPC + b] = 1.0
    for c in range(n_cores):
        in_maps[c]["pmask"] = pmask[c]

    plan = dict(
        G=G, F=F, FH=FH, FO=FO, BPC=BPC, RPC=RPC, NP=NP, HALF=HALF,
        TT=TT, IDXCOLS=IDXCOLS, NCALLS=NCALLS, MAXG=MAXG, groups=groups,
        NCHUNK=NCHUNK, CHB=CHB,
        n_cores=n_cores,
        has_b1=has_b1,
        has_b2=has_b2,
        has_bfc=bool(np.any(np.asarray(bfc))),
        MAXCT=MAXG,  # for test harness prints
    )
    return plan, in_maps


# --------------------------------------------------------------------------
# Bass program builder (identical on all cores).
# --------------------------------------------------------------------------

def build(plan, debug=False):
    dt = mybir.dt
    G, F, FH, FO = plan["G"], plan["F"], plan["FH"], plan["FO"]
    BPC, RPC, NP, HALF = plan["BPC"], plan["RPC"], plan["NP"], plan["HALF"]
    TT, IDXCOLS, NCALLS = plan["TT"], plan["IDXCOLS"], plan["NCALLS"]
    MAXG = plan["MAXG"]
    NCHUNK, CHB = plan["NCHUNK"], plan["CHB"]
    groups = plan["groups"]
    n_cores = plan["n_cores"]
    KC = F // P  # k-chunks for the transforms (2)
    FCK = (3 * FH) // P  # k-chunks for the FC (6)
    has_bias = plan["has_b1"] or plan["has_b2"]
    SP = bool(int(os.environ.get("SP", "0")))

    nc = bacc.Bacc("TRN2", target_bir_lowering=False, debug=debug,
                   num_devices=n_cores)

    def din(name, shape, dtype):
        return nc.dram_tensor(name, shape, dtype, kind="ExternalInput").ap()

    xg_d = din("xg", [P, TT * F], dt.bfloat16)
    xself_d = din("xself", [P, BPC * F], dt.bfloat16)
    w1_d = din("w1", [F, FH], dt.bfloat16)
    w2_d = din("w2", [FH, FH], dt.bfloat16)
    wfc_d = din("wfc", [3 * FH, FO], dt.bfloat16)
    b1r_d = din("b1r", [1, FH], dt.bfloat16)
    b2r_d = din("b2r", [1, FH], dt.bfloat16)
    bfcr_d = din("bfcr", [1, FO], dt.bfloat16)
    if has_bias:
        sqdeg_d = din("sqdeg", [1, RPC], dt.bfloat16)
    dinv_d = din("dinv", [P, BPC], dt.float32)
    gidx_d = din("gidx", [P, IDXCOLS], dt.int16)
    gslot_d = din("gslot", [P, TT], dt.float32)
    pm_d = din("pm", [P, BPC * G], dt.bfloat16)
    pmask_d = din("pmask", [P, G * BPC], dt.bfloat16)
    recip_d = din("recip", [G, 1], dt.float32)
    out_d = nc.dram_tensor("out", [G, FO], dt.float32,
                           kind="ExternalOutput").ap()

    rg = [list(range(n_cores))]

    from contextlib import ExitStack
    with tile.TileContext(nc) as tc, ExitStack() as ctx:
        const = ctx.enter_context(tc.tile_pool(name="const", bufs=1))
        dram = ctx.enter_context(tc.tile_pool(name="dram", bufs=1, space="DRAM"))
        tfpsum = ctx.enter_context(tc.tile_pool(name="tfpsum", bufs=2, space="PSUM"))
        aggpsum = ctx.enter_context(tc.tile_pool(name="aggpsum", bufs=2, space="PSUM"))
        tpsum = ctx.enter_context(tc.tile_pool(name="tpsum", bufs=2, space="PSUM"))
        spsum = ctx.enter_context(tc.tile_pool(name="spsum", bufs=1, space="PSUM"))
        fcpsum = ctx.enter_context(tc.tile_pool(name="fcpsum", bufs=1, space="PSUM"))
        msgp = ctx.enter_context(tc.tile_pool(name="msgp", bufs=2))
        btp = ctx.enter_context(tc.tile_pool(name="btp", bufs=8))
        hp = ctx.enter_context(tc.tile_pool(name="hp", bufs=4))
        htp = ctx.enter_context(tc.tile_pool(name="htp", bufs=6))
        tailp = ctx.enter_context(tc.tile_pool(name="tailp", bufs=1))

        # ---------------- constants into SBUF ----------------
        def cload(tag, dram_ap, shape, dtype):
            t = const.tile(shape, dtype, tag=tag)
            nc.sync.dma_start(out=t[:], in_=dram_ap)
            return t

        w_sb = []
        for tag, d in (("w1", w1_d), ("w2", w2_d)):
            t = const.tile([P, KC * FH], dt.bfloat16, tag=tag)
            for c in range(KC):
                nc.sync.dma_start(out=t[:, c * FH:(c + 1) * FH],
                                  in_=d[c * P:(c + 1) * P, :])
            w_sb.append(t)
        wfc_sb = const.tile([P, FCK * FO], dt.bfloat16, tag="wfc")
        for c in range(FCK):
            nc.sync.dma_start(out=wfc_sb[:, c * FO:(c + 1) * FO],
                              in_=wfc_d[c * P:(c + 1) * P, :])
        b1r_sb = cload("b1r", b1r_d, [1, FH], dt.bfloat16)
        b2r_sb = cload("b2r", b2r_d, [1, FH], dt.bfloat16)
        bfcr_sb = cload("bfcr", bfcr_d, [1, FO], dt.bfloat16)
        if has_bias:
            sqdeg_sb = cload("sqdeg", sqdeg_d, [1, RPC], dt.bfloat16)
        dinv_sb = cload("dinv", dinv_d, [P, BPC], dt.float32)
        gidx_sb = cload("gidx", gidx_d, [P, IDXCOLS], dt.int16)
        gslot_sb = cload("gslot", gslot_d, [P, TT], dt.float32)
        xself_sb = cload("xself", xself_d, [P, BPC * F], dt.bfloat16)
        pm_sb = cload("pm", pm_d, [P, BPC * G], dt.bfloat16)
        pmask_sb = cload("pmask", pmask_d, [P, G * BPC], dt.bfloat16)
        recip_sb = cload("recip", recip_d, [G, 1], dt.float32)

        iota_sb = const.tile([P, P], dt.float32, tag="iota")
        nc.gpsimd.iota(out=iota_sb[:], pattern=[[1, P]], base=0,
                       channel_multiplier=0,
                       allow_small_or_imprecise_dtypes=True)
        iotac_sb = const.tile([P, 1], dt.float32, tag="iotac")
        nc.gpsimd.iota(out=iotac_sb[:], pattern=[[0, 1]], base=0,
                       channel_multiplier=1,
                       allow_small_or_imprecise_dtypes=True)
        ident_sb = const.tile([P, P], dt.bfloat16, tag="ident")
        nc.vector.tensor_tensor(out=ident_sb[:],
                                in0=iotac_sb[:].to_broadcast([P, P]),
                                in1=iota_sb[:],
                                op=mybir.AluOpType.is_equal)
        ones_sb = const.tile([1, G], dt.bfloat16, tag="ones")
        nc.gpsimd.memset(ones_sb[:], 1.0)
        tbl2own = const.tile([P, BPC * FH], dt.bfloat16, tag="tbl2own")
        blockmax = const.tile([P, KC * BPC], dt.bfloat16, tag="bmax")

        # DRAM bounce buffers for collectives
        ag_in = dram.tile([RPC, FH], dt.bfloat16, name="agin", tag="agin")
        ag_outs = [dram.tile([n_cores * CHB * P, FH], dt.bfloat16,
                             name=f"agout{k}", tag=f"agout{k}",
                             addr_space="Shared")
                   for k in range(NCHUNK)]
        ars_in = dram.tile([G, FH], dt.float32, tag="arsin")
        ars_out = dram.tile([G, FH], dt.float32, tag="arsout",
                            addr_space="Shared")
        arm_in = dram.tile([P, KC * G], dt.bfloat16, tag="armin")
        arm_out = dram.tile([P, KC * G], dt.bfloat16, tag="armout",
                            addr_space="Shared")
        warm_in = dram.tile([1, 16], dt.float32, tag="warmin")
        warm_out = dram.tile([1, 16], dt.float32, tag="warmout",
                             addr_space="Shared")

        Copy = mybir.ActivationFunctionType.Copy
        Relu = mybir.ActivationFunctionType.Relu

        # warm up the collectives stack (entry barrier etc.) during layer 1
        warm_sb = tailp.tile([1, 16], dt.float32, tag="warm_sb")
        nc.gpsimd.memset(warm_sb[:], 0.0)
        nc.sync.dma_start(out=warm_in[:], in_=warm_sb[:])
        nc.gpsimd.collective_compute(
            "AllReduce", mybir.AluOpType.add,
            ins=[warm_in[:].opt()], outs=[warm_out[:].opt()],
            replica_groups=rg)

        def build_onehot(gt):
            # tensor_tensor broadcast is ~4-8x faster than tensor_scalar here
            bt = btp.tile([P, P], dt.bfloat16, tag="bt")
            nc.vector.tensor_tensor(
                out=bt[:],
                in0=gslot_sb[:, gt:gt + 1].to_broadcast([P, P]),
                in1=iota_sb[:],
                op=mybir.AluOpType.is_equal)
            return bt

        # ---------------- layer 1: scatter pre-gathered X, then transform --
        done_blocks = 0
        next_chunk = 0
        for grp in groups:
            nt_g = grp["ntiles"]
            if nt_g > 0:
                xgt = msgp.tile([P, MAXG * F], dt.bfloat16, tag="msg")
                a = grp["tstart"] * F
                nc.sync.dma_start(out=xgt[:, :nt_g * F],
                                  in_=xg_d[:, a:a + nt_g * F])
            for b in grp["blocks"]:
                tiles_b = [call["tstart"] + t0 + t
                           for call in grp["calls"]
                           for (bb, t0, tcnt) in call["blocks"] if bb == b
                           for t in range(tcnt)]
                nmm = len(tiles_b)
                ps = aggpsum.tile([P, FH], dt.float32, tag="aggps")
                nc.tensor.matmul(
                    out=ps[:], lhsT=ident_sb[:],
                    rhs=xself_sb[:, b * F:(b + 1) * F],
                    start=True,
                    stop=(nmm == 0 and not plan["has_b1"]))
                for k, gt in enumerate(tiles_b):
                    loc = gt - grp["tstart"]
                    bt = build_onehot(gt)
                    nc.tensor.matmul(
                        out=ps[:], lhsT=bt[:],
                        rhs=xgt[:, loc * F:(loc + 1) * F],
                        start=False,
                        stop=(k + 1 == nmm) and not plan["has_b1"])
                if plan["has_b1"]:
                    nc.tensor.matmul(
                        out=ps[:],
                        lhsT=sqdeg_sb[:, b * P:(b + 1) * P],
                        rhs=b1r_sb[:],
                        start=False, stop=True)

                # epilogue: t1 = dinv*S; h1 = relu(t1 @ W1);
                # table2 = dinv * (h1 @ W2)
                t1 = hp.tile([P, FH], dt.bfloat16, tag="t1")
                nc.scalar.activation(out=t1[:], in_=ps[:], func=Copy,
                                     scale=dinv_sb[:, b:b + 1])
                ps2 = tfpsum.tile([P, FH], dt.float32, tag="tfps")
                for c in range(KC):
                    tp = tpsum.tile([P, P], dt.bfloat16, tag="tp")
                    nc.tensor.transpose(out=tp[:],
                                        in_=t1[:, c * P:(c + 1) * P],
                                        identity=ident_sb[:])
                    ht = htp.tile([P, P], dt.bfloat16, tag="ht")
                    nc.vector.tensor_copy(out=ht[:], in_=tp[:])
                    nc.tensor.matmul(out=ps2[:], lhsT=ht[:],
                                     rhs=w_sb[0][:, c * FH:(c + 1) * FH],
                                     start=(c == 0), stop=(c == KC - 1))
                h1 = hp.tile([P, FH], dt.bfloat16, tag="h1")
                nc.scalar.activation(out=h1[:], in_=ps2[:], func=Relu)
                ps3 = tfpsum.tile([P, FH], dt.float32, tag="tfps")
                for c in range(KC):
                    tp = tpsum.tile([P, P], dt.bfloat16, tag="tp")
                    nc.tensor.transpose(out=tp[:],
                                        in_=h1[:, c * P:(c + 1) * P],
                                        identity=ident_sb[:])
                    ht = htp.tile([P, P], dt.bfloat16, tag="ht")
                    nc.vector.tensor_copy(out=ht[:], in_=tp[:])
                    nc.tensor.matmul(out=ps3[:], lhsT=ht[:],
                                     rhs=w_sb[1][:, c * FH:(c + 1) * FH],
                                     start=(c == 0), stop=(c == KC - 1))
                nc.scalar.activation(out=tbl2own[:, b * FH:(b + 1) * FH],
                                     in_=ps3[:], func=Copy,
                                     scale=dinv_sb[:, b:b + 1])
                nc.sync.dma_start(out=ag_in[b * P:(b + 1) * P, :],
                                  in_=tbl2own[:, b * FH:(b + 1) * FH])
                done_blocks += 1
                # fire AllGather chunks as soon as their blocks are written
                while (next_chunk < NCHUNK
                       and done_blocks >= (next_chunk + 1) * CHB):
                    r0 = next_chunk * CHB * P
                    r1 = (next_chunk + 1) * CHB * P
                    nc.gpsimd.collective_compute(
                        "AllGather", mybir.AluOpType.bypass,
                        ins=[ag_in[r0:r1, :].opt()],
                        outs=[ag_outs[next_chunk][:].opt()],
                        replica_groups=rg)
                    next_chunk += 1

        # ---------------- layer 2: gather + scatter + pooling epilogue ----
        sums_ps = spsum.tile([G, FH], dt.float32, tag="sums")

        for grp in groups:
            mb = None
            nt_g = grp["ntiles"]
            if nt_g > 0:
                mb = msgp.tile([P, MAXG * FH], dt.bfloat16, tag="msg")
                for call in grp["calls"]:
                    h, nt = call["h"], call["ntiles"]
                    loc0 = call["tstart"] - grp["tstart"]
                    out_ap = mb[:, loc0 * FH:(loc0 + nt) * FH].rearrange(
                        "p (t e) -> p t e", e=FH)
                    table_ap = (ag_outs[h][:] if NCHUNK == 2
                                else ag_outs[0][h * HALF:(h + 1) * HALF, :])
                    nc.gpsimd.dma_gather(
                        out_ap=out_ap,
                        in_ap=table_ap,
                        idxs_ap=gidx_sb[:, call["idx_off"]:
                                        call["idx_off"] + nt * 8],
                        num_idxs=nt * P,
                        num_idxs_reg=nt * P,
                        elem_size=FH,
                        single_packet=SP)
            for b in grp["blocks"]:
                tiles_b = [call["tstart"] + t0 + t
                           for call in grp["calls"]
                           for (bb, t0, tcnt) in call["blocks"] if bb == b
                           for t in range(tcnt)]
                nmm = len(tiles_b)
                ps = aggpsum.tile([P, FH], dt.float32, tag="aggps")
                nc.tensor.matmul(
                    out=ps[:], lhsT=ident_sb[:],
                    rhs=tbl2own[:, b * FH:(b + 1) * FH],
                    start=True,
                    stop=(nmm == 0 and not plan["has_b2"]))
                for k, gt in enumerate(tiles_b):
                    loc = gt - grp["tstart"]
                    bt = build_onehot(gt)
                    nc.tensor.matmul(
                        out=ps[:], lhsT=bt[:],
                        rhs=mb[:, loc * FH:(loc + 1) * FH],
                        start=False,
                        stop=(k + 1 == nmm) and not plan["has_b2"])
                if plan["has_b2"]:
                    nc.tensor.matmul(
                        out=ps[:],
                        lhsT=sqdeg_sb[:, b * P:(b + 1) * P],
                        rhs=b2r_sb[:],
                        start=False, stop=True)
                # epilogue: h2 = relu(dinv * ps); pooling contributions
                h2 = hp.tile([P, FH], dt.bfloat16, tag="h2")
                nc.scalar.activation(out=h2[:], in_=ps[:], func=Relu,
                                     scale=dinv_sb[:, b:b + 1])
                nc.tensor.matmul(out=sums_ps[:],
                                 lhsT=pm_sb[:, b * G:(b + 1) * G],
                                 rhs=h2[:],
                                 start=(b == 0), stop=(b == BPC - 1))
                for c in range(KC):
                    tp = tpsum.tile([P, P], dt.bfloat16, tag="tp")
                    nc.tensor.transpose(out=tp[:],
                                        in_=h2[:, c * P:(c + 1) * P],
                                        identity=ident_sb[:])
                    nc.vector.tensor_reduce(
                        out=blockmax[:, c * BPC + b:c * BPC + b + 1],
                        in_=tp[:], axis=mybir.AxisListType.X,
                        op=mybir.AluOpType.max)

        # ---------------- pooling tail ----------------
        sums_sb = tailp.tile([G, FH], dt.float32, tag="sums_sb")
        nc.vector.tensor_copy(out=sums_sb[:], in_=sums_ps[:])
        nc.sync.dma_start(out=ars_in[:], in_=sums_sb[:])
        nc.gpsimd.collective_compute(
            "AllReduce", mybir.AluOpType.add,
            ins=[ars_in[:].opt()], outs=[ars_out[:].opt()],
            replica_groups=rg)
        # per-graph LOCAL max from this core's block maxima via masks
        mxT_loc = tailp.tile([P, KC * G], dt.bfloat16, tag="mxT_loc")
        mtmp = tailp.tile([P, BPC], dt.bfloat16, tag="mtmp")
        for g in range(G):
            for c in range(KC):
                nc.vector.tensor_tensor(
                    out=mtmp[:], in0=blockmax[:, c * BPC:(c + 1) * BPC],
                    in1=pmask_sb[:, g * BPC:(g + 1) * BPC],
                    op=mybir.AluOpType.mult)
                nc.vector.tensor_reduce(
                    out=mxT_loc[:, c * G + g:c * G + g + 1], in_=mtmp[:],
                    axis=mybir.AxisListType.X, op=mybir.AluOpType.max)
        nc.sync.dma_start(out=arm_in[:], in_=mxT_loc[:])
        nc.gpsimd.collective_compute(
            "AllReduce", mybir.AluOpType.max,
            ins=[arm_in[:].opt()], outs=[arm_out[:].opt()],
            replica_groups=rg)

        gsums = tailp.tile([G, FH], dt.float32, tag="gsums")
        nc.sync.dma_start(out=gsums[:], in_=ars_out[:])
        mxT = tailp.tile([P, KC * G], dt.bfloat16, tag="mxT")
        nc.sync.dma_start(out=mxT[:], in_=arm_out[:])

        # mean / sums in bf16, transposed to feature-major for the FC
        mean_sb = tailp.tile([G, FH], dt.bfloat16, tag="mean")
        nc.vector.tensor_scalar(out=mean_sb[:], in0=gsums[:],
                                scalar1=recip_sb[:], scalar2=None,
                                op0=mybir.AluOpType.mult)
        sums_bf = tailp.tile([G, FH], dt.bfloat16, tag="sumsbf")
        nc.vector.tensor_copy(out=sums_bf[:], in_=gsums[:])
        meanT = tailp.tile([P, KC * G], dt.bfloat16, tag="meanT")
        sumsT = tailp.tile([P, KC * G], dt.bfloat16, tag="sumsT")
        for src, dst_t in ((mean_sb, meanT), (sums_bf, sumsT)):
            for c in range(KC):
                tp = tpsum.tile([P, P], dt.bfloat16, tag="tp")
                nc.tensor.transpose(out=tp[:, :G],
                                    in_=src[:, c * P:(c + 1) * P],
                                    identity=ident_sb[:G, :G])
                nc.vector.tensor_copy(out=dst_t[:, c * G:(c + 1) * G],
                                      in_=tp[:, :G])

        # final FC: out = [mean | max | sums] @ Wfc + bfc
        fc_ps = fcpsum.tile([G, FO], dt.float32, tag="fc")
        gT = [meanT, mxT, sumsT]
        k = 0
        for part in range(3):
            for c in range(KC):
                nc.tensor.matmul(
                    out=fc_ps[:], lhsT=gT[part][:, c * G:(c + 1) * G],
                    rhs=wfc_sb[:, k * FO:(k + 1) * FO],
                    start=(k == 0),
                    stop=(k == FCK - 1) and not plan["has_bfc"])
                k += 1
        if plan["has_bfc"]:
            nc.tensor.matmul(out=fc_ps[:], lhsT=ones_sb[:], rhs=bfcr_sb[:],
                             start=False, stop=True)
        out_sb = tailp.tile([G, FO], dt.float32, tag="out_sb")
        nc.vector.tensor_copy(out=out_sb[:], in_=fc_ps[:])
        nc.sync.dma_start(out=out_d[:], in_=out_sb[:])

    nc.compile()
    return nc


# --------------------------------------------------------------------------
# Entry point for the grading harness.
# --------------------------------------------------------------------------

def kernel(x, edge_index, batch, n_graphs, W1, b1, W2, b2, Wfc, bfc,
           **_unused):
    plan, in_maps = preprocess(x, edge_index, batch, n_graphs,
                               W1, b1, W2, b2, Wfc, bfc)
    nc = build(plan)
    res = run_bass_kernel_spmd(nc, in_maps, core_ids=list(range(NCORES)))
    out = np.asarray(res.results[0]["out"], np.float32)
    return out


# revision 32
# speedup vs baseline: 2.3441x; 1.0456x over previous
"""Trainium2 Bass kernel for a 2-layer GCN (EnhancedGNN) with triple global
pooling and a final FC, run SPMD across 8 NeuronCores.

Strategy v3:
  - Nodes are re-ordered into graph-pure 128-row blocks, padded per graph,
    sharded contiguously across the 8 cores (dst / data parallel). Within
    each core, blocks are sorted by edge count so the max-over-cores tile
    padding shrinks (the SPMD program uses per-(block,half) maxima).
  - Layer 1 does NOT gather on device: since X is a replicated input and
    scatter/transform commute ((B^T Xg) @ W1 == B^T (Xg @ W1)), the host
    pre-lays-out dinv-scaled X rows in message-tile order and the device
    streams them with affine DMA, scatters them into per-dst-block PSUM via
    one-hot matmuls, then applies W1 (and W2 to build the layer-2 table).
  - Self-loop messages fold in as an identity matmul against the block's
    own rows (X rows for layer 1, own layer-2 table rows for layer 2).
  - The layer-2 table AllGather is chunked (NCHUNK pieces) so all but the
    last chunk overlap layer-1 compute; a tiny warmup AllReduce at program
    start absorbs the collectives' entry barrier.
  - Layer 2: per-(block,half) dma_gather calls with trailing -1 padding and
    a per-core runtime num_idxs_reg, so the GpSimd Q7 descriptor generation
    (the measured bottleneck) only pays for real edges on each core.
  - Pooling: per-graph sums via one-hot matmul; per-graph max accumulated
    incrementally per block (vector max) during layer 2; AllReduce(add/max);
    the tiny FC runs redundantly on every core.

The kernel program is identical on all 8 cores (SPMD); all per-core
differences live in the input data. Structure constants (tile counts etc.)
are maxima over cores so the program is uniform.
"""

import os
import numpy as np
import ml_dtypes

import concourse.bass as bass
import concourse.tile as tile
from concourse import bacc, mybir
from concourse.bass_utils import run_bass_kernel_spmd

P = 128
NCORES = 8
GROUP_NBLK = 2  # dst blocks per gather/stream group

BF16 = ml_dtypes.bfloat16


def _cdiv(a, b):
    return -(-a // b)


# --------------------------------------------------------------------------
# Host-side preprocessing: sharding, edge grouping, auxiliary tensors.
# --------------------------------------------------------------------------

def preprocess(x, edge_index, batch, n_graphs, W1, b1, W2, b2, Wfc, bfc,
               n_cores=NCORES):
    x = np.asarray(x, np.float32)
    ei = np.asarray(edge_index, np.int64)
    batch = np.asarray(batch, np.int64)
    G = int(n_graphs)
    N = x.shape[0]
    F = x.shape[1]
    FH = W1.shape[1]
    FO = Wfc.shape[1]
    assert F == FH, "kernel assumes F_IN == F_HID"

    # degrees (dst side, + self loop), as in the reference
    deg = np.bincount(ei[1], minlength=N).astype(np.float32) + 1.0
    dinv = 1.0 / np.sqrt(deg)
    sqdeg = np.sqrt(deg)

    # --- graph-padded node ordering (pure blocks) ---
    cnt = np.bincount(batch, minlength=G).astype(np.int64)  # nodes per graph
    blocks_g = _cdiv(cnt, P)  # 0 for empty graphs
    total_blocks = int(blocks_g.sum())
    total_blocks_padded = _cdiv(total_blocks, n_cores) * n_cores
    BPC = total_blocks_padded // n_cores
    RPC = BPC * P
    NP = total_blocks_padded * P
    HALF = NP // 2
    assert HALF <= 32768, f"table half {HALF} exceeds int16 index range"

    # AllGather chunking: 2 pieces, each one gather half (chunk == half), so
    # each chunk is a single Shared DRAM tile written by exactly one AG and
    # read as exactly one gather source region.
    NCHUNK = 2 if BPC % 2 == 0 else 1
    CHB = BPC // NCHUNK          # blocks per chunk (per core)
    CHR = CHB * P                # rows per chunk (per core)

    blk_start = np.concatenate([[0], np.cumsum(blocks_g)])  # per graph
    row_start = blk_start * P
    first_node = np.concatenate([[0], np.cumsum(cnt)])[:-1]

    def layout(block_perm=None):
        """node -> padded row, with optional per-core block permutation.

        block_perm[c][l] = pre-layout local block that lands at local slot l.
        """
        pre_pos = row_start[batch] + (np.arange(N) - first_node[batch])
        if block_perm is None:
            return pre_pos
        pre_blk = pre_pos // P
        c_of = pre_blk // BPC
        l_of = pre_blk % BPC
        # inverse: where does pre-local-block l of core c go?
        inv = np.zeros((n_cores, BPC), np.int64)
        for c in range(n_cores):
            inv[c, block_perm[c]] = np.arange(BPC)
        new_blk = c_of * BPC + inv[c_of, l_of]
        return new_blk * P + (pre_pos % P)

    # pass 1: preliminary layout to measure per-(core, block, half) counts
    new_pos = layout()
    es0, ed0 = new_pos[ei[0]], new_pos[ei[1]]
    cnt3p = np.zeros((n_cores, BPC, 2), np.int64)
    np.add.at(cnt3p, (ed0 // RPC, (ed0 % RPC) // P, es0 // HALF), 1)
    # sort blocks within each core by max-half count (desc) to align maxima
    key = np.maximum(cnt3p[:, :, 0], cnt3p[:, :, 1])
    block_perm = np.argsort(-key, axis=1)
    new_pos = layout(block_perm)

    row2node = np.full(NP, -1, np.int64)
    row2node[new_pos] = np.arange(N)
    real = row2node >= 0

    # per padded row data
    x_pad = np.zeros((NP, F), np.float32)
    x_pad[real] = x[row2node[real]]
    dinv_pad = np.ones(NP, np.float32)
    dinv_pad[real] = dinv[row2node[real]]
    sqdeg_pad = np.zeros(NP, np.float32)
    sqdeg_pad[real] = sqdeg[row2node[real]]
    xs_pad = (x_pad * dinv_pad[:, None]).astype(BF16)
    g_of_block = np.full(total_blocks_padded, -1, np.int64)
    gb = np.where(real, batch[np.clip(row2node, 0, N - 1)], -1)
    for j in range(total_blocks_padded):
        blkg = gb[j * P:(j + 1) * P]
        blkg = blkg[blkg >= 0]
        if blkg.size:
            g_of_block[j] = blkg[0]

    # --- edges WITHOUT self loops (self loops folded via identity matmul) ---
    es = new_pos[ei[0]]
    ed = new_pos[ei[1]]
    core = ed // RPC
    blk = (ed % RPC) // P
    slot = ed % P
    # gather-table address of a src row under the chunked AllGather layout:
    # chunk k holds [8 cores x CHR rows] at offset k*8*CHR.
    src_c = es // RPC
    src_lr = es % RPC
    src_k = src_lr // CHR
    gaddr = src_k * (n_cores * CHR) + src_c * CHR + (src_lr % CHR)
    half = gaddr // HALF
    lsrc = gaddr - half * HALF

    cnt3 = np.zeros((n_cores, BPC, 2), np.int64)
    np.add.at(cnt3, (core, blk, half), 1)
    T = np.max(_cdiv(cnt3, P), axis=0)  # [BPC, 2] tiles, uniform across cores

    # call / group structure: one gather call per (group, half) spanning the
    # group's blocks — few large calls amortize the ~µs fixed cost per call.
    blocks_groups = [list(range(s, min(s + GROUP_NBLK, BPC)))
                     for s in range(0, BPC, GROUP_NBLK)]
    groups = []
    tt = 0
    idxcols = 0
    ncalls = 0
    tile_of = np.zeros((BPC, 2), np.int64)
    for gblocks in blocks_groups:
        calls = []
        g_t0 = tt
        for h in (0, 1):
            ntiles = int(sum(T[b, h] for b in gblocks))
            if ntiles == 0:
                continue
            blocks_in_call = []
            t0 = 0
            for b in gblocks:
                tile_of[b, h] = tt + t0
                blocks_in_call.append((b, t0, int(T[b, h])))
                t0 += int(T[b, h])
            calls.append(dict(h=h, ntiles=ntiles, tstart=tt,
                              idx_off=idxcols, call_id=ncalls,
                              blocks=blocks_in_call))
            tt += ntiles
            idxcols += ntiles * 8
            ncalls += 1
        groups.append(dict(blocks=gblocks, calls=calls,
                           tstart=g_t0, ntiles=tt - g_t0))
    TT = tt
    IDXCOLS = idxcols
    NCALLS = max(ncalls, 1)
    MAXG = max((g["ntiles"] for g in groups), default=1)

    # --- per-core edge arrays in tile order ---
    order = np.lexsort((lsrc, half, blk, core))
    so_lsrc, so_slot, so_src = lsrc[order], slot[order], es[order]
    run_start = np.zeros((n_cores, BPC, 2), np.int64)
    flat_cnt = cnt3.reshape(-1)
    np.cumsum(flat_cnt[:-1], out=run_start.reshape(-1)[1:])

    idxflat = np.zeros((n_cores, TT * P), np.int16)
    slotflat = np.full((n_cores, TT * P), 255.0, np.float32)
    srcflat = np.zeros((n_cores, TT * P), np.int64)
    validflat = np.zeros((n_cores, TT * P), bool)
    for c in range(n_cores):
        for b in range(BPC):
            for h in (0, 1):
                if T[b, h] == 0:
                    continue
                n = int(cnt3[c, b, h])
                if n == 0:
                    continue
                s0 = int(run_start[c, b, h])
                o = int(tile_of[b, h]) * P
                idxflat[c, o:o + n] = so_lsrc[s0:s0 + n].astype(np.int16)
                slotflat[c, o:o + n] = so_slot[s0:s0 + n]
                srcflat[c, o:o + n] = so_src[s0:s0 + n]
                validflat[c, o:o + n] = True

    # wrap-16 + replicate-to-128 index layout, call-local (layer 2 gather)
    gidx = np.zeros((n_cores, P, IDXCOLS), np.int16)
    for g in groups:
        for call in g["calls"]:
            a = call["tstart"] * P
            nt = call["ntiles"]
            region = idxflat[:, a:a + nt * P]           # [NC, nt*128]
            arr = region.reshape(n_cores, nt * 8, 16)   # i -> (i//16, i%16)
            arr = arr.transpose(0, 2, 1)                # [NC, 16, cols]
            gidx[:, :, call["idx_off"]:call["idx_off"] + nt * 8] = (
                np.tile(arr, (1, 8, 1)))
    gslot = slotflat.reshape(n_cores, TT, P).transpose(0, 2, 1).copy()

    # --- layer-1 pre-gathered message tiles (dinv_src * x[src]) ---
    xg = np.zeros((n_cores, P, TT * F), BF16)
    for c in range(n_cores):
        rows = xs_pad[srcflat[c]]                       # [TT*P, F] bf16
        rows[~validflat[c]] = 0
        xg[c] = rows.reshape(TT, P, F).transpose(1, 0, 2).reshape(P, TT * F)
    xself = np.zeros((n_cores, P, BPC * F), BF16)
    for c in range(n_cores):
        r0 = c * RPC
        xself[c] = (xs_pad[r0:r0 + RPC]
                    .reshape(BPC, P, F).transpose(1, 0, 2).reshape(P, BPC * F))

    # --- pooling helpers ---
    rows_i = np.arange(NP)
    rcore = rows_i // RPC
    rblk = (rows_i % RPC) // P
    rslot = rows_i % P
    pm = np.zeros((n_cores, P, BPC * G), BF16)
    pm[rcore[real], rslot[real], rblk[real] * G + gb[real]] = 1.0
    recip = (1.0 / np.maximum(cnt, 1.0)).astype(np.float32).reshape(G, 1)

    has_b1 = bool(np.any(np.asarray(b1)))
    has_b2 = bool(np.any(np.asarray(b2)))

    # --- per-core input maps ---
    in_maps = []
    for c in range(n_cores):
        r0, r1 = c * RPC, (c + 1) * RPC
        m = {
            "xg": xg[c],
            "xself": xself[c],
            "w1": np.asarray(W1, np.float32).astype(BF16),
            "w2": np.asarray(W2, np.float32).astype(BF16),
            "wfc": np.asarray(Wfc, np.float32).astype(BF16),
            "b1r": np.asarray(b1, np.float32).reshape(1, FH).astype(BF16),
            "b2r": np.asarray(b2, np.float32).reshape(1, FH).astype(BF16),
            "bfcr": np.asarray(bfc, np.float32).reshape(1, FO).astype(BF16),
            "dinv": np.ascontiguousarray(
                dinv_pad[r0:r1].reshape(BPC, P).T).astype(np.float32),
            "gidx": gidx[c],
            "gslot": gslot[c],
            "pm": pm[c],
            "recip": recip,
        }
        if has_b1 or has_b2:
            m["sqdeg"] = sqdeg_pad[r0:r1].reshape(1, RPC).astype(BF16)
        in_maps.append(m)

    # graph id of each local block per core (host constant for the program;
    # same structure across cores is NOT required for data, but the program
    # needs a uniform instruction stream -> use per-core data via masks).
    # Incremental max uses g_of_block of THIS core; but the program must be
    # uniform, so instead we use a per-core "maxcol" input: column index in
    # mxT_loc for each block (or a dump column G for trash blocks).
    maxcol = np.zeros((n_cores, BPC), np.int64)
    for c in range(n_cores):
        for b in range(BPC):
            g = g_of_block[c * BPC + b]
            maxcol[c, b] = g if g >= 0 else G
    # maxcol differs per core -> cannot be baked into the (uniform) program.
    # Instead supply a per-core one-hot routing matrix per block is overkill;
    # we use a [P, BPC] bf16 "bsel" input: bsel[:, b] is all-ones if block b
    # is real, else zeros, and a per-core int map is impossible -- so we
    # instead accumulate per-BLOCK maxima into a [P, KC*BPC] buffer (uniform)
    # and do the masked per-graph reduction in the tail as before, but with
    # the mask multiply fused to KC*G vector ops over [P, BPC] tiles.
    pmask = np.zeros((n_cores, P, G * BPC), BF16)
    for c in range(n_cores):
        for b in range(BPC):
            g = g_of_block[c * BPC + b]
            if g >= 0:
                pmask[c, :, g * BPC + b] = 1.0
    for c in range(n_cores):
        in_maps[c]["pmask"] = pmask[c]

    plan = dict(
        G=G, F=F, FH=FH, FO=FO, BPC=BPC, RPC=RPC, NP=NP, HALF=HALF,
        TT=TT, IDXCOLS=IDXCOLS, NCALLS=NCALLS, MAXG=MAXG, groups=groups,
        NCHUNK=NCHUNK, CHB=CHB,
        n_cores=n_cores,
        has_b1=has_b1,
        has_b2=has_b2,
        has_bfc=bool(np.any(np.asarray(bfc))),
        MAXCT=MAXG,  # for test harness prints
    )
    return plan, in_maps


# --------------------------------------------------------------------------
# Bass program builder (identical on all cores).
# --------------------------------------------------------------------------

def build(plan, debug=False):
    dt = mybir.dt
    G, F, FH, FO = plan["G"], plan["F"], plan["FH"], plan["FO"]
    BPC, RPC, NP, HALF = plan["BPC"], plan["RPC"], plan["NP"], plan["HALF"]
    TT, IDXCOLS, NCALLS = plan["TT"], plan["IDXCOLS"], plan["NCALLS"]
    MAXG = plan["MAXG"]
    NCHUNK, CHB = plan["NCHUNK"], plan["CHB"]
    groups = plan["groups"]
    n_cores = plan["n_cores"]
    KC = F // P  # k-chunks for the transforms (2)
    FCK = (3 * FH) // P  # k-chunks for the FC (6)
    has_bias = plan["has_b1"] or plan["has_b2"]
    SP = bool(int(os.environ.get("SP", "0")))

    nc = bacc.Bacc("TRN2", target_bir_lowering=False, debug=debug,
                   num_devices=n_cores)

    def din(name, shape, dtype):
        return nc.dram_tensor(name, shape, dtype, kind="ExternalInput").ap()

    xg_d = din("xg", [P, TT * F], dt.bfloat16)
    xself_d = din("xself", [P, BPC * F], dt.bfloat16)
    w1_d = din("w1", [F, FH], dt.bfloat16)
    w2_d = din("w2", [FH, FH], dt.bfloat16)
    wfc_d = din("wfc", [3 * FH, FO], dt.bfloat16)
    b1r_d = din("b1r", [1, FH], dt.bfloat16)
    b2r_d = din("b2r", [1, FH], dt.bfloat16)
    bfcr_d = din("bfcr", [1, FO], dt.bfloat16)
    if has_bias:
        sqdeg_d = din("sqdeg", [1, RPC], dt.bfloat16)
    dinv_d = din("dinv", [P, BPC], dt.float32)
    gidx_d = din("gidx", [P, IDXCOLS], dt.int16)
    gslot_d = din("gslot", [P, TT], dt.float32)
    pm_d = din("pm", [P, BPC * G], dt.bfloat16)
    pmask_d = din("pmask", [P, G * BPC], dt.bfloat16)
    recip_d = din("recip", [G, 1], dt.float32)
    out_d = nc.dram_tensor("out", [G, FO], dt.float32,
                           kind="ExternalOutput").ap()

    rg = [list(range(n_cores))]

    from contextlib import ExitStack
    with tile.TileContext(nc) as tc, ExitStack() as ctx:
        const = ctx.enter_context(tc.tile_pool(name="const", bufs=1))
        dram = ctx.enter_context(tc.tile_pool(name="dram", bufs=1, space="DRAM"))
        tfpsum = ctx.enter_context(tc.tile_pool(name="tfpsum", bufs=2, space="PSUM"))
        aggpsum = ctx.enter_context(tc.tile_pool(name="aggpsum", bufs=2, space="PSUM"))
        tpsum = ctx.enter_context(tc.tile_pool(name="tpsum", bufs=2, space="PSUM"))
        spsum = ctx.enter_context(tc.tile_pool(name="spsum", bufs=1, space="PSUM"))
        fcpsum = ctx.enter_context(tc.tile_pool(name="fcpsum", bufs=1, space="PSUM"))
        msgp = ctx.enter_context(tc.tile_pool(name="msgp", bufs=2))
        msgp2 = ctx.enter_context(tc.tile_pool(name="msgp2", bufs=3))
        btp = ctx.enter_context(tc.tile_pool(name="btp", bufs=8))
        hp = ctx.enter_context(tc.tile_pool(name="hp", bufs=4))
        htp = ctx.enter_context(tc.tile_pool(name="htp", bufs=6))
        tailp = ctx.enter_context(tc.tile_pool(name="tailp", bufs=1))

        # ---------------- constants into SBUF ----------------
        def cload(tag, dram_ap, shape, dtype):
            t = const.tile(shape, dtype, tag=tag)
            nc.sync.dma_start(out=t[:], in_=dram_ap)
            return t

        w_sb = []
        for tag, d in (("w1", w1_d), ("w2", w2_d)):
            t = const.tile([P, KC * FH], dt.bfloat16, tag=tag)
            for c in range(KC):
                nc.sync.dma_start(out=t[:, c * FH:(c + 1) * FH],
                                  in_=d[c * P:(c + 1) * P, :])
            w_sb.append(t)
        wfc_sb = const.tile([P, FCK * FO], dt.bfloat16, tag="wfc")
        for c in range(FCK):
            nc.sync.dma_start(out=wfc_sb[:, c * FO:(c + 1) * FO],
                              in_=wfc_d[c * P:(c + 1) * P, :])
        b1r_sb = cload("b1r", b1r_d, [1, FH], dt.bfloat16)
        b2r_sb = cload("b2r", b2r_d, [1, FH], dt.bfloat16)
        bfcr_sb = cload("bfcr", bfcr_d, [1, FO], dt.bfloat16)
        if has_bias:
            sqdeg_sb = cload("sqdeg", sqdeg_d, [1, RPC], dt.bfloat16)
        dinv_sb = cload("dinv", dinv_d, [P, BPC], dt.float32)
        gidx_sb = cload("gidx", gidx_d, [P, IDXCOLS], dt.int16)
        gslot_sb = cload("gslot", gslot_d, [P, TT], dt.float32)
        xself_sb = cload("xself", xself_d, [P, BPC * F], dt.bfloat16)
        pm_sb = cload("pm", pm_d, [P, BPC * G], dt.bfloat16)
        pmask_sb = cload("pmask", pmask_d, [P, G * BPC], dt.bfloat16)
        recip_sb = cload("recip", recip_d, [G, 1], dt.float32)

        iota_sb = const.tile([P, P], dt.float32, tag="iota")
        nc.gpsimd.iota(out=iota_sb[:], pattern=[[1, P]], base=0,
                       channel_multiplier=0,
                       allow_small_or_imprecise_dtypes=True)
        iotac_sb = const.tile([P, 1], dt.float32, tag="iotac")
        nc.gpsimd.iota(out=iotac_sb[:], pattern=[[0, 1]], base=0,
                       channel_multiplier=1,
                       allow_small_or_imprecise_dtypes=True)
        ident_sb = const.tile([P, P], dt.bfloat16, tag="ident")
        nc.vector.tensor_tensor(out=ident_sb[:],
                                in0=iotac_sb[:].to_broadcast([P, P]),
                                in1=iota_sb[:],
                                op=mybir.AluOpType.is_equal)
        ones_sb = const.tile([1, G], dt.bfloat16, tag="ones")
        nc.gpsimd.memset(ones_sb[:], 1.0)
        tbl2own = const.tile([P, BPC * FH], dt.bfloat16, tag="tbl2own")
        blockmax = const.tile([P, KC * BPC], dt.bfloat16, tag="bmax")

        # DRAM bounce buffers for collectives
        ag_in = dram.tile([RPC, FH], dt.bfloat16, name="agin", tag="agin")
        ag_outs = [dram.tile([n_cores * CHB * P, FH], dt.bfloat16,
                             name=f"agout{k}", tag=f"agout{k}",
                             addr_space="Shared")
                   for k in range(NCHUNK)]
        ars_in = dram.tile([G, FH], dt.float32, tag="arsin")
        ars_out = dram.tile([G, FH], dt.float32, tag="arsout",
                            addr_space="Shared")
        arm_in = dram.tile([P, KC * G], dt.bfloat16, tag="armin")
        arm_out = dram.tile([P, KC * G], dt.bfloat16, tag="armout",
                            addr_space="Shared")
        warm_in = dram.tile([1, 16], dt.float32, tag="warmin")
        warm_out = dram.tile([1, 16], dt.float32, tag="warmout",
                             addr_space="Shared")

        Copy = mybir.ActivationFunctionType.Copy
        Relu = mybir.ActivationFunctionType.Relu

        # warm up the collectives stack (entry barrier etc.) during layer 1
        warm_sb = tailp.tile([1, 16], dt.float32, tag="warm_sb")
        nc.gpsimd.memset(warm_sb[:], 0.0)
        nc.sync.dma_start(out=warm_in[:], in_=warm_sb[:])
        nc.gpsimd.collective_compute(
            "AllReduce", mybir.AluOpType.add,
            ins=[warm_in[:].opt()], outs=[warm_out[:].opt()],
            replica_groups=rg)

        def build_onehot(gt):
            # tensor_tensor broadcast is ~4-8x faster than tensor_scalar here
            bt = btp.tile([P, P], dt.bfloat16, tag="bt")
            nc.vector.tensor_tensor(
                out=bt[:],
                in0=gslot_sb[:, gt:gt + 1].to_broadcast([P, P]),
                in1=iota_sb[:],
                op=mybir.AluOpType.is_equal)
            return bt

        # ---------------- layer 1: scatter pre-gathered X, then transform --
        done_blocks = 0
        next_chunk = 0
        for grp in groups:
            nt_g = grp["ntiles"]
            if nt_g > 0:
                xgt = msgp.tile([P, MAXG * F], dt.bfloat16, tag="msg")
                a = grp["tstart"] * F
                nc.sync.dma_start(out=xgt[:, :nt_g * F],
                                  in_=xg_d[:, a:a + nt_g * F])
            for b in grp["blocks"]:
                tiles_b = [call["tstart"] + t0 + t
                           for call in grp["calls"]
                           for (bb, t0, tcnt) in call["blocks"] if bb == b
                           for t in range(tcnt)]
                nmm = len(tiles_b)
                ps = aggpsum.tile([P, FH], dt.float32, tag="aggps")
                nc.tensor.matmul(
                    out=ps[:], lhsT=ident_sb[:],
                    rhs=xself_sb[:, b * F:(b + 1) * F],
                    start=True,
                    stop=(nmm == 0 and not plan["has_b1"]))
                for k, gt in enumerate(tiles_b):
                    loc = gt - grp["tstart"]
                    bt = build_onehot(gt)
                    nc.tensor.matmul(
                        out=ps[:], lhsT=bt[:],
                        rhs=xgt[:, loc * F:(loc + 1) * F],
                        start=False,
                        stop=(k + 1 == nmm) and not plan["has_b1"])
                if plan["has_b1"]:
                    nc.tensor.matmul(
                        out=ps[:],
                        lhsT=sqdeg_sb[:, b * P:(b + 1) * P],
                        rhs=b1r_sb[:],
                        start=False, stop=True)

                # epilogue: t1 = dinv*S; h1 = relu(t1 @ W1);
                # table2 = dinv * (h1 @ W2)
                t1 = hp.tile([P, FH], dt.bfloat16, tag="t1")
                nc.scalar.activation(out=t1[:], in_=ps[:], func=Copy,
                                     scale=dinv_sb[:, b:b + 1])
                ps2 = tfpsum.tile([P, FH], dt.float32, tag="tfps")
                for c in range(KC):
                    tp = tpsum.tile([P, P], dt.bfloat16, tag="tp")
                    nc.tensor.transpose(out=tp[:],
                                        in_=t1[:, c * P:(c + 1) * P],
                                        identity=ident_sb[:])
                    ht = htp.tile([P, P], dt.bfloat16, tag="ht")
                    nc.vector.tensor_copy(out=ht[:], in_=tp[:])
                    nc.tensor.matmul(out=ps2[:], lhsT=ht[:],
                                     rhs=w_sb[0][:, c * FH:(c + 1) * FH],
                                     start=(c == 0), stop=(c == KC - 1))
                h1 = hp.tile([P, FH], dt.bfloat16, tag="h1")
                nc.scalar.activation(out=h1[:], in_=ps2[:], func=Relu)
                ps3 = tfpsum.tile([P, FH], dt.float32, tag="tfps")
                for c in range(KC):
                    tp = tpsum.tile([P, P], dt.bfloat16, tag="tp")
                    nc.tensor.transpose(out=tp[:],
                                        in_=h1[:, c * P:(c + 1) * P],
                                        identity=ident_sb[:])
                    ht = htp.tile([P, P], dt.bfloat16, tag="ht")
                    nc.vector.tensor_copy(out=ht[:], in_=tp[:])
                    nc.tensor.matmul(out=ps3[:], lhsT=ht[:],
                                     rhs=w_sb[1][:, c * FH:(c + 1) * FH],
                                     start=(c == 0), stop=(c == KC - 1))
                nc.scalar.activation(out=tbl2own[:, b * FH:(b + 1) * FH],
                                     in_=ps3[:], func=Copy,
                                     scale=dinv_sb[:, b:b + 1])
                nc.sync.dma_start(out=ag_in[b * P:(b + 1) * P, :],
                                  in_=tbl2own[:, b * FH:(b + 1) * FH])
                done_blocks += 1
                # fire AllGather chunks as soon as their blocks are written
                while (next_chunk < NCHUNK
                       and done_blocks >= (next_chunk + 1) * CHB):
                    r0 = next_chunk * CHB * P
                    r1 = (next_chunk + 1) * CHB * P
                    nc.gpsimd.collective_compute(
                        "AllGather", mybir.AluOpType.bypass,
                        ins=[ag_in[r0:r1, :].opt()],
                        outs=[ag_outs[next_chunk][:].opt()],
                        replica_groups=rg)
                    next_chunk += 1

        # ---------------- layer 2: gather + scatter + pooling epilogue ----
        sums_ps = spsum.tile([G, FH], dt.float32, tag="sums")

        for grp in groups:
            mb = None
            nt_g = grp["ntiles"]
            if nt_g > 0:
                mb = msgp2.tile([P, MAXG * FH], dt.bfloat16, tag="msg2")
                for call in grp["calls"]:
                    h, nt = call["h"], call["ntiles"]
                    loc0 = call["tstart"] - grp["tstart"]
                    out_ap = mb[:, loc0 * FH:(loc0 + nt) * FH].rearrange(
                        "p (t e) -> p t e", e=FH)
                    table_ap = (ag_outs[h][:] if NCHUNK == 2
                                else ag_outs[0][h * HALF:(h + 1) * HALF, :])
                    nc.gpsimd.dma_gather(
                        out_ap=out_ap,
                        in_ap=table_ap,
                        idxs_ap=gidx_sb[:, call["idx_off"]:
                                        call["idx_off"] + nt * 8],
                        num_idxs=nt * P,
                        num_idxs_reg=nt * P,
                        elem_size=FH,
                        single_packet=SP)
            for b in grp["blocks"]:
                tiles_b = [call["tstart"] + t0 + t
                           for call in grp["calls"]
                           for (bb, t0, tcnt) in call["blocks"] if bb == b
                           for t in range(tcnt)]
                nmm = len(tiles_b)
                ps = aggpsum.tile([P, FH], dt.float32, tag="aggps")
                nc.tensor.matmul(
                    out=ps[:], lhsT=ident_sb[:],
                    rhs=tbl2own[:, b * FH:(b + 1) * FH],
                    start=True,
                    stop=(nmm == 0 and not plan["has_b2"]))
                for k, gt in enumerate(tiles_b):
                    loc = gt - grp["tstart"]
                    bt = build_onehot(gt)
                    nc.tensor.matmul(
                        out=ps[:], lhsT=bt[:],
                        rhs=mb[:, loc * FH:(loc + 1) * FH],
                        start=False,
                        stop=(k + 1 == nmm) and not plan["has_b2"])
                if plan["has_b2"]:
                    nc.tensor.matmul(
                        out=ps[:],
                        lhsT=sqdeg_sb[:, b * P:(b + 1) * P],
                        rhs=b2r_sb[:],
                        start=False, stop=True)
                # epilogue: h2 = relu(dinv * ps); pooling contributions
                h2 = hp.tile([P, FH], dt.bfloat16, tag="h2")
                nc.scalar.activation(out=h2[:], in_=ps[:], func=Relu,
                                     scale=dinv_sb[:, b:b + 1])
                nc.tensor.matmul(out=sums_ps[:],
                                 lhsT=pm_sb[:, b * G:(b + 1) * G],
                                 rhs=h2[:],
                                 start=(b == 0), stop=(b == BPC - 1))
                for c in range(KC):
                    tp = tpsum.tile([P, P], dt.bfloat16, tag="tp")
                    nc.tensor.transpose(out=tp[:],
                                        in_=h2[:, c * P:(c + 1) * P],
                                        identity=ident_sb[:])
                    nc.vector.tensor_reduce(
                        out=blockmax[:, c * BPC + b:c * BPC + b + 1],
                        in_=tp[:], axis=mybir.AxisListType.X,
                        op=mybir.AluOpType.max)

        # ---------------- pooling tail ----------------
        sums_sb = tailp.tile([G, FH], dt.float32, tag="sums_sb")
        nc.vector.tensor_copy(out=sums_sb[:], in_=sums_ps[:])
        nc.sync.dma_start(out=ars_in[:], in_=sums_sb[:])
        nc.gpsimd.collective_compute(
            "AllReduce", mybir.AluOpType.add,
            ins=[ars_in[:].opt()], outs=[ars_out[:].opt()],
            replica_groups=rg)
        # per-graph LOCAL max from this core's block maxima via masks
        mxT_loc = tailp.tile([P, KC * G], dt.bfloat16, tag="mxT_loc")
        mtmp = tailp.tile([P, BPC], dt.bfloat16, tag="mtmp")
        for g in range(G):
            for c in range(KC):
                nc.vector.tensor_tensor(
                    out=mtmp[:], in0=blockmax[:, c * BPC:(c + 1) * BPC],
                    in1=pmask_sb[:, g * BPC:(g + 1) * BPC],
                    op=mybir.AluOpType.mult)
                nc.vector.tensor_reduce(
                    out=mxT_loc[:, c * G + g:c * G + g + 1], in_=mtmp[:],
                    axis=mybir.AxisListType.X, op=mybir.AluOpType.max)
        nc.sync.dma_start(out=arm_in[:], in_=mxT_loc[:])
        nc.gpsimd.collective_compute(
            "AllReduce", mybir.AluOpType.max,
            ins=[arm_in[:].opt()], outs=[arm_out[:].opt()],
            replica_groups=rg)

        gsums = tailp.tile([G, FH], dt.float32, tag="gsums")
        nc.sync.dma_start(out=gsums[:], in_=ars_out[:])
        mxT = tailp.tile([P, KC * G], dt.bfloat16, tag="mxT")
        nc.sync.dma_start(out=mxT[:], in_=arm_out[:])

        # mean / sums in bf16, transposed to feature-major for the FC
        mean_sb = tailp.tile([G, FH], dt.bfloat16, tag="mean")
        nc.vector.tensor_scalar(out=mean_sb[:], in0=gsums[:],
                                scalar1=recip_sb[:], scalar2=None,
                                op0=mybir.AluOpType.mult)
        sums_bf = tailp.tile([G, FH], dt.bfloat16, tag="sumsbf")
        nc.vector.tensor_copy(out=sums_bf[:], in_=gsums[:])
        meanT = tailp.tile([P, KC * G], dt.bfloat16, tag="meanT")
        sumsT = tailp.tile([P, KC * G], dt.bfloat16, tag="sumsT")
        for src, dst_t in ((mean_sb, meanT), (sums_bf, sumsT)):
            for c in range(KC):
                tp = tpsum.tile([P, P], dt.bfloat16, tag="tp")
                nc.tensor.transpose(out=tp[:, :G],
                                    in_=src[:, c * P:(c + 1) * P],
                                    identity=ident_sb[:G, :G])
                nc.vector.tensor_copy(out=dst_t[:, c * G:(c + 1) * G],
                                      in_=tp[:, :G])

        # final FC: out = [mean | max | sums] @ Wfc + bfc
        fc_ps = fcpsum.tile([G, FO], dt.float32, tag="fc")
        gT = [meanT, mxT, sumsT]
        k = 0
        for part in range(3):
            for c in range(KC):
                nc.tensor.matmul(
                    out=fc_ps[:], lhsT=gT[part][:, c * G:(c + 1) * G],
                    rhs=wfc_sb[:, k * FO:(k + 1) * FO],
                    start=(k == 0),
                    stop=(k == FCK - 1) and not plan["has_bfc"])
                k += 1
        if plan["has_bfc"]:
            nc.tensor.matmul(out=fc_ps[:], lhsT=ones_sb[:], rhs=bfcr_sb[:],
                             start=False, stop=True)
        out_sb = tailp.tile([G, FO], dt.float32, tag="out_sb")
        nc.vector.tensor_copy(out=out_sb[:], in_=fc_ps[:])
        nc.sync.dma_start(out=out_d[:], in_=out_sb[:])

    nc.compile()
    return nc


# --------------------------------------------------------------------------
# Entry point for the grading harness.
# --------------------------------------------------------------------------

def kernel(x, edge_index, batch, n_graphs, W1, b1, W2, b2, Wfc, bfc,
           **_unused):
    plan, in_maps = preprocess(x, edge_index, batch, n_graphs,
                               W1, b1, W2, b2, Wfc, bfc)
    nc = build(plan)
    res = run_bass_kernel_spmd(nc, in_maps, core_ids=list(range(NCORES)))
    out = np.asarray(res.results[0]["out"], np.float32)
    return out
